# revision 11
# baseline (speedup 1.0000x reference)
"""AttentionTSSA Trainium2 kernel.

Sharding: data-parallel over batch. B=8 -> one batch element per NeuronCore,
zero collectives. Host slices inputs / stacks outputs.

Per-core math (x: [N=4096, D=1024], heads h=16, head dim d=64):
  w[n, c]   = x @ W_qkv.T                   (c = hd flattened head*64+dd)
  s[c]      = sum_n w^2                     (col norms squared)
  logits[h,n] = temp[h] * sum_dd w^2[hd,n] / max(s[hd], 1e-24)
  Pi        = softmax_h(logits)
  dots[c]   = (sum_n Pi[h,n] * w^2[c,n]) / (sum_n Pi[h,n] + 1e-8)
  attn[c]   = 1 / (1 + dots)
  y         = (-(w * Pi_bcast) * attn_bcast) @ W_out.T + b_out

On-chip layout: w stored column-major [c(part), n(free)] as 8 tiles
[128, 4096] bf16, so every sum_n is a free-axis reduce and both big
matmuls (f32r, full PE rate) need no big transposes beyond x itself
(PE-transposed per chunk).
"""

import sys

sys.path.insert(0, "/opt/trn_rl_repo")

import numpy as np
import concourse.bacc as bacc
import concourse.tile as tile
from concourse import mybir
from concourse.bass_utils import run_bass_kernel_spmd
from concourse.masks import make_identity

F32 = mybir.dt.float32
F32R = mybir.dt.float32r
BF16 = mybir.dt.bfloat16
MUL = mybir.AluOpType.mult
ADD = mybir.AluOpType.add

B, N, D = 8, 4096, 1024
H, HD = 16, 64
P = 128
NT = D // P          # 8 col-partition tiles
CH = 512             # n-chunk
NCH = N // CH        # 8 chunks
MS = CH // P         # 4 n-subtiles per chunk


def build():
    nc = bacc.Bacc()
    x_t = nc.dram_tensor("x", [N, D], F32, kind="ExternalInput")
    wq_t = nc.dram_tensor("wqT", [D, D], F32R, kind="ExternalInput")    # W_qkv.T
    wo_t = nc.dram_tensor("woT", [D, D], F32R, kind="ExternalInput")    # W_out.T
    temp_t = nc.dram_tensor("temp", [H, 1], F32, kind="ExternalInput")
    sel_t = nc.dram_tensor("sel", [NT, H, P], F32, kind="ExternalInput")
    selr_t = nc.dram_tensor("selr", [NT, H, P], F32R, kind="ExternalInput")
    bias_t = nc.dram_tensor("bout", [1, D], F32R, kind="ExternalInput")
    y_t = nc.dram_tensor("y", [N, D], F32, kind="ExternalOutput")

    with tile.TileContext(nc) as tc:
        with (
            tc.tile_pool(name="consts", bufs=1) as consts,
            tc.tile_pool(name="wmat", bufs=1) as wmat,
            tc.tile_pool(name="wsb", bufs=1) as wsb,
            tc.tile_pool(name="small", bufs=1) as small,
        ):
            # ---------- constants ----------
            ident = consts.tile([P, P], F32)
            make_identity(nc, ident)
            temp_sb = consts.tile([H, 1], F32)
            nc.sync.dma_start(out=temp_sb, in_=temp_t[:, :])
            bias_r = consts.tile([1, D], F32R)
            nc.sync.dma_start(out=bias_r, in_=bias_t[:, :])
            ones16 = consts.tile([H, 1], F32)
            nc.vector.memset(ones16, 1.0)
            ones1x16 = consts.tile([1, H], F32)
            nc.vector.memset(ones1x16, 1.0)
            ones1x128 = consts.tile([1, P], F32)
            nc.vector.memset(ones1x128, 1.0)
            ones1x128_r = consts.tile([1, P], F32R)
            nc.vector.tensor_copy(ones1x128_r, ones1x128)

            # per-tile selectors (host constant): Sel01[t][j, p] = 1 iff j == 2t + p//64
            sel_f32 = []
            sel_r = []
            for t in range(NT):
                sf = consts.tile([H, P], F32, tag=f"self{t}", name=f"self{t}")
                nc.sync.dma_start(out=sf, in_=sel_t[t, :, :])
                sr = consts.tile([H, P], F32R, tag=f"selr{t}", name=f"selr{t}")
                nc.sync.dma_start(out=sr, in_=selr_t[t, :, :])
                sel_f32.append(sf)
                sel_r.append(sr)

            # weights (host pre-transposed); wq and wo share one slot (bufs=1):
            # wo is DMA'd after phase A frees wq
            wq_sb = wmat.tile([P, NT, D], F32R, tag="wm")
            for k in range(NT):
                nc.sync.dma_start(out=wq_sb[:, k, :], in_=wq_t[k * P : (k + 1) * P, :])

            # persistent big tensors
            w_tiles = [wsb.tile([P, N], BF16, tag=f"w{t}", name=f"w{t}") for t in range(NT)]
            s_strip = [small.tile([P, NCH], F32, tag=f"ss{t}", name=f"ss{t}") for t in range(NT)]
            d_strip = [small.tile([P, NCH], F32, tag=f"ds{t}", name=f"ds{t}") for t in range(NT)]

            # ---------- phase A: w = x @ WqkvT, s = sum_n w^2 ----------
            with (
                tc.tile_pool(name="achunk", bufs=2) as achunk,
                tc.tile_pool(name="scrA", bufs=3) as scrA,
                tc.tile_pool(name="psA", bufs=2, space="PSUM") as psA,
                tc.tile_pool(name="psTP", bufs=2, space="PSUM") as psTP,
            ):
                for c in range(NCH):
                    cs = slice(c * CH, (c + 1) * CH)
                    x_raw = achunk.tile([P, MS, D], F32, tag="xraw")
                    nc.sync.dma_start(
                        out=x_raw,
                        in_=x_t[cs, :].rearrange("(m p) i -> p m i", p=P),
                    )
                    xT = achunk.tile([P, NT, CH], F32R, tag="xT", bufs=1)
                    for k in range(NT):
                        tp_ps = psTP.tile([P, CH], F32, tag="tp")
                        for m in range(MS):
                            nc.tensor.transpose(
                                tp_ps[:, m * P : (m + 1) * P],
                                x_raw[:, m, k * P : (k + 1) * P],
                                ident,
                            )
                        nc.scalar.copy(out=xT[:, k, :], in_=tp_ps)
                    for t in range(NT):
                        w_ps = psA.tile([P, CH], F32, tag="mm1")
                        for k in range(NT):
                            nc.tensor.matmul(
                                w_ps,
                                wq_sb[:, k, t * P : (t + 1) * P],
                                xT[:, k, :],
                                start=(k == 0),
                                stop=(k == NT - 1),
                            )
                        nc.scalar.copy(out=w_tiles[t][:, cs], in_=w_ps)
                        junk = scrA.tile([P, CH], BF16, tag="junkA")
                        nc.vector.scalar_tensor_tensor(
                            out=junk,
                            in0=w_tiles[t][:, cs],
                            scalar=1.0,
                            in1=w_tiles[t][:, cs],
                            op0=MUL,
                            op1=MUL,
                            accum_out=s_strip[t][:, c : c + 1],
                        )

            # W_out.T load (reuses wq's slot; overlaps softmax phases)
            wo_sb = wmat.tile([P, NT, D], F32R, tag="wm")
            for k in range(NT):
                nc.sync.dma_start(out=wo_sb[:, k, :], in_=wo_t[k * P : (k + 1) * P, :])

            # softmax-side pool opens only after phase A frees its space
            soft = tc.alloc_tile_pool(name="soft", bufs=1)

            # ---------- stats 1: inv_temp, L_big ----------
            lbig = []
            with tc.tile_pool(name="psS1", bufs=2, space="PSUM") as psS1:
                for t in range(NT):
                    s_all = small.tile([P, 1], F32, tag=f"sall{t}")
                    nc.vector.reduce_sum(s_all, s_strip[t], axis=mybir.AxisListType.X)
                    nc.vector.tensor_scalar_max(out=s_all, in0=s_all, scalar1=1e-24)
                    rcp = small.tile([P, 1], F32, tag=f"rcp{t}")
                    nc.vector.reciprocal(rcp, s_all)
                    tb_ps = psS1.tile([P, 1], F32, tag="tb")
                    nc.tensor.matmul(tb_ps, sel_f32[t], temp_sb, start=True, stop=True)
                    inv_t = small.tile([P, 1], F32, tag=f"invt{t}")
                    nc.vector.tensor_mul(inv_t, rcp, tb_ps)
                    lb = small.tile([P, H], BF16, tag=f"lbig{t}")
                    nc.vector.memset(lb, 0.0)
                    nc.vector.tensor_copy(lb[0:HD, 2 * t : 2 * t + 1], inv_t[0:HD, :])
                    nc.vector.tensor_copy(
                        lb[HD:P, 2 * t + 1 : 2 * t + 2], inv_t[HD:P, :]
                    )
                    lbig.append(lb)

            # ---------- phase L: logits[h, n] ----------
            logits = soft.tile([H, N], F32, tag="soft")
            with (
                tc.tile_pool(name="scrL", bufs=3) as scrL,
                tc.tile_pool(name="psL", bufs=2, space="PSUM") as psL,
            ):
                for c in range(NCH):
                    cs = slice(c * CH, (c + 1) * CH)
                    lg_ps = psL.tile([H, CH], F32, tag="lg")
                    for t in range(NT):
                        w2t = scrL.tile([P, CH], BF16, tag="w2t")
                        nc.scalar.activation(
                            out=w2t,
                            in_=w_tiles[t][:, cs],
                            func=mybir.ActivationFunctionType.Square,
                        )
                        nc.tensor.matmul(
                            lg_ps, lbig[t], w2t, start=(t == 0), stop=(t == NT - 1)
                        )
                    nc.vector.tensor_copy(logits[:, cs], lg_ps)

            # ---------- softmax over h + sumPi ----------
            epool = tc.alloc_tile_pool(name="epool", bufs=1)
            e_hn = epool.tile([H, N], F32, tag="ehn")
            nc.scalar.activation(
                out=e_hn, in_=logits, func=mybir.ActivationFunctionType.Exp
            )
            pi_hn = soft.tile([H, N], F32R, tag="soft")
            sumpi_strip = small.tile([H, NCH], F32, tag="spstrip")
            sume_row = small.tile([1, N], F32, tag="sumerow")
            r_row = small.tile([1, N], F32, tag="rrow")
            with tc.tile_pool(name="psSM", bufs=2, space="PSUM") as psSM:
                for c in range(NCH):
                    cs = slice(c * CH, (c + 1) * CH)
                    se_ps = psSM.tile([1, CH], F32, tag="se")
                    nc.tensor.matmul(se_ps, ones16, e_hn[:, cs], start=True, stop=True)
                    nc.scalar.copy(out=sume_row[:, cs], in_=se_ps)
                nc.vector.reciprocal(r_row, sume_row)
                for c in range(NCH):
                    cs = slice(c * CH, (c + 1) * CH)
                    rb_ps = psSM.tile([H, CH], F32, tag="rb")
                    nc.tensor.matmul(
                        rb_ps, ones1x16, r_row[:, cs], start=True, stop=True
                    )
                    nc.vector.scalar_tensor_tensor(
                        out=pi_hn[:, cs],
                        in0=e_hn[:, cs],
                        scalar=1.0,
                        in1=rb_ps,
                        op0=MUL,
                        op1=MUL,
                        accum_out=sumpi_strip[:, c : c + 1],
                    )

            epool.release()

            sumpi = small.tile([H, 1], F32, tag="sumpi")
            nc.vector.reduce_sum(sumpi, sumpi_strip, axis=mybir.AxisListType.X)
            nc.vector.tensor_scalar_add(out=sumpi, in0=sumpi, scalar1=1e-8)
            ispi = small.tile([H, 1], F32, tag="ispi")
            nc.vector.reciprocal(ispi, sumpi)

            # ---------- phase B1: dots ----------
            with (
                tc.tile_pool(name="scrB", bufs=3) as scrB,
                tc.tile_pool(name="psB1", bufs=2, space="PSUM") as psB1,
            ):
                for c in range(NCH):
                    cs = slice(c * CH, (c + 1) * CH)
                    for t in range(NT):
                        pib_ps = psB1.tile([P, CH], F32, tag="pib")
                        nc.tensor.matmul(
                            pib_ps, sel_r[t], pi_hn[:, cs], start=True, stop=True
                        )
                        w2t = scrB.tile([P, CH], BF16, tag="w2b")
                        nc.scalar.activation(
                            out=w2t,
                            in_=w_tiles[t][:, cs],
                            func=mybir.ActivationFunctionType.Square,
                        )
                        junk = scrB.tile([P, CH], BF16, tag="junkB")
                        nc.vector.scalar_tensor_tensor(
                            out=junk,
                            in0=w2t,
                            scalar=1.0,
                            in1=pib_ps,
                            op0=MUL,
                            op1=MUL,
                            accum_out=d_strip[t][:, c : c + 1],
                        )

            # ---------- stats 2: attn, fold -attn into WoutT rows ----------
            with tc.tile_pool(name="psS2", bufs=2, space="PSUM") as psS2:
                for t in range(NT):
                    isp_ps = psS2.tile([P, 1], F32, tag="isp")
                    nc.tensor.matmul(isp_ps, sel_f32[t], ispi, start=True, stop=True)
                    dots = small.tile([P, 1], F32, tag=f"dots{t}")
                    nc.vector.reduce_sum(dots, d_strip[t], axis=mybir.AxisListType.X)
                    nc.vector.tensor_mul(dots, dots, isp_ps)
                    nc.vector.tensor_scalar_add(out=dots, in0=dots, scalar1=1.0)
                    attn = small.tile([P, 1], F32, tag=f"attn{t}")
                    nc.vector.reciprocal(attn, dots)
                    nc.vector.tensor_scalar_mul(out=attn, in0=attn, scalar1=-1.0)
                    nc.vector.tensor_scalar_mul(
                        out=wo_sb[:, t, :],
                        in0=wo_sb[:, t, :].bitcast(F32),
                        scalar1=attn,
                    )

            # ---------- phase B2: u = w * Pi_b ; y = u.T @ Wout' + b ----------
            with (
                tc.tile_pool(name="uch", bufs=1) as uch,
                tc.tile_pool(name="och", bufs=1) as och,
                tc.tile_pool(name="psB2", bufs=2, space="PSUM") as psB2,
                tc.tile_pool(name="psMM2", bufs=2, space="PSUM") as psMM2,
            ):
                for c in range(NCH):
                    cs = slice(c * CH, (c + 1) * CH)
                    u_ch = uch.tile([P, NT, CH], F32R, tag="u")
                    for t in range(NT):
                        pib_ps = psB2.tile([P, CH], F32, tag="pib2")
                        nc.tensor.matmul(
                            pib_ps, sel_r[t], pi_hn[:, cs], start=True, stop=True
                        )
                        nc.vector.tensor_mul(
                            u_ch[:, t, :], w_tiles[t][:, cs], pib_ps
                        )
                    outf = och.tile([P, MS, D], F32, tag="outf")
                    for m in range(MS):
                        for oh in range(2):
                            os_ = slice(oh * CH, (oh + 1) * CH)
                            f_ps = psMM2.tile([P, CH], F32, tag="mm2")
                            for t in range(NT):
                                nc.tensor.matmul(
                                    f_ps,
                                    u_ch[:, t, m * P : (m + 1) * P],
                                    wo_sb[:, t, os_],
                                    start=(t == 0),
                                    stop=False,
                                )
                            nc.tensor.matmul(
                                f_ps,
                                ones1x128_r,
                                bias_r[:, os_],
                                start=False,
                                stop=True,
                            )
                            nc.scalar.copy(out=outf[:, m, os_], in_=f_ps)
                    nc.sync.dma_start(
                        out=y_t[cs, :].rearrange("(m p) i -> p m i", p=P),
                        in_=outf,
                    )
            soft.release()

    if not nc.is_finalized():
        nc.finalize()
    return nc


_NC_CACHE = None
_LAST_IN_MAPS = None
_RUNNER = None


def _make_runner(nc, n_cores):
    """Like bass2jax.run_bass_via_pjrt but with the jitted callable cached,
    so repeat calls don't re-trace/re-compile the XLA wrapper."""
    import jax
    from jax.experimental.shard_map import shard_map
    from jax.sharding import Mesh, PartitionSpec
    from concourse import mybir as _mybir
    from concourse.bass2jax import (
        _bass_exec_p,
        install_neuronx_cc_hook,
        partition_id_tensor,
    )

    install_neuronx_cc_hook()

    partition_name = nc.partition_id_tensor.name if nc.partition_id_tensor else None
    in_names, out_names, out_avals, zero_outs = [], [], [], []
    for alloc in nc.m.functions[0].allocations:
        if not isinstance(alloc, _mybir.MemoryLocationSet):
            continue
        name = alloc.memorylocations[0].name
        if alloc.kind == "ExternalInput":
            if name != partition_name:
                in_names.append(name)
        elif alloc.kind == "ExternalOutput":
            shape = tuple(alloc.tensor_shape)
            dtype = _mybir.dt.np(alloc.dtype)
            out_names.append(name)
            out_avals.append(jax.core.ShapedArray(shape, dtype))
            zero_outs.append(np.zeros(shape, dtype))
    n_params = len(in_names)
    n_outs = len(out_names)
    all_in_names = in_names + out_names + (
        [partition_name] if partition_name else []
    )
    donate = tuple(range(n_params, n_params + n_outs))

    def _body(*args):
        operands = list(args)
        if partition_name is not None:
            operands.append(partition_id_tensor())
        outs = _bass_exec_p.bind(
            *operands,
            out_avals=tuple(out_avals),
            in_names=tuple(all_in_names),
            out_names=tuple(out_names),
            lowering_input_output_aliases=(),
            sim_require_finite=True,
            sim_require_nnan=True,
            nc=nc,
        )
        return tuple(outs)

    devices = jax.devices()[:n_cores]
    mesh = Mesh(np.asarray(devices), ("core",))
    in_specs = (PartitionSpec("core"),) * (n_params + n_outs)
    out_specs = (PartitionSpec("core"),) * n_outs
    sharded = jax.jit(
        shard_map(
            _body, mesh=mesh, in_specs=in_specs, out_specs=out_specs, check_rep=False
        ),
        donate_argnums=donate,
        keep_unused=True,
    )

    def run(in_maps):
        concat_in = [
            np.concatenate([np.asarray(m[name]) for m in in_maps], axis=0)
            for name in in_names
        ]
        concat_zeros = [
            np.zeros((n_cores * z.shape[0], *z.shape[1:]), z.dtype)
            for z in zero_outs
        ]
        out_arrs = sharded(*concat_in, *concat_zeros)
        return {
            name: np.asarray(out_arrs[i]).reshape(n_cores, *out_avals[i].shape)
            for i, name in enumerate(out_names)
        }

    run.sharded = sharded
    run.meta = (in_names, out_names, out_avals, n_params, n_outs)
    return run


def kernel(x, W_qkv, temp, W_out, b_out):
    global _NC_CACHE, _RUNNER
    if _NC_CACHE is None:
        _NC_CACHE = build()
        _RUNNER = _make_runner(_NC_CACHE, B)
    nc = _NC_CACHE

    x = np.asarray(x, dtype=np.float32)
    wqT = np.ascontiguousarray(np.asarray(W_qkv, dtype=np.float32).T)
    woT = np.ascontiguousarray(np.asarray(W_out, dtype=np.float32).T)
    temp = np.ascontiguousarray(np.asarray(temp, dtype=np.float32).reshape(H, 1))
    bout = np.ascontiguousarray(np.asarray(b_out, dtype=np.float32).reshape(1, D))

    sel = np.zeros((NT, H, P), dtype=np.float32)
    for t in range(NT):
        sel[t, 2 * t, 0:HD] = 1.0
        sel[t, 2 * t + 1, HD:P] = 1.0

    core_ids = list(range(B))
    in_maps = [
        {"x": np.ascontiguousarray(x[i]), "wqT": wqT, "woT": woT,
         "temp": temp, "bout": bout, "sel": sel, "selr": sel}
        for i in core_ids
    ]
    global _LAST_IN_MAPS
    _LAST_IN_MAPS = in_maps
    out = _RUNNER(in_maps)
    return out["y"]


if __name__ == "__main__":
    rng = np.random.default_rng(0)
    x = rng.standard_normal((B, N, D), dtype=np.float32)
    W_qkv = (rng.standard_normal((D, D), dtype=np.float32) * 0.02).astype(np.float32)
    temp = np.ones((H, 1), dtype=np.float32)
    W_out = (rng.standard_normal((D, D), dtype=np.float32) * 0.02).astype(np.float32)
    b_out = np.zeros((D,), dtype=np.float32)
    y = kernel(x=x, W_qkv=W_qkv, temp=temp, W_out=W_out, b_out=b_out)
    print("kernel ran, y shape", y.shape, "mean abs", np.abs(y).mean())


# revision 17
# speedup vs baseline: 1.0768x; 1.0768x over previous
"""AttentionTSSA Trainium2 kernel.

Sharding: data-parallel over batch. B=8 -> one batch element per NeuronCore,
zero collectives. Host slices inputs / stacks outputs.

Per-core math (x: [N=4096, D=1024], heads h=16, head dim d=64):
  w[n, c]   = x @ W_qkv.T                   (c = hd flattened head*64+dd)
  s[c]      = sum_n w^2                     (col norms squared)
  logits[h,n] = temp[h] * sum_dd w^2[hd,n] / max(s[hd], 1e-24)
  Pi        = softmax_h(logits)
  dots[c]   = (sum_n Pi[h,n] * w^2[c,n]) / (sum_n Pi[h,n] + 1e-8)
  attn[c]   = 1 / (1 + dots)
  y         = (-(w * Pi_bcast) * attn_bcast) @ W_out.T + b_out

On-chip layout: w stored column-major [c(part), n(free)] as 8 tiles
[128, 4096] bf16, so every sum_n is a free-axis reduce and both big
matmuls (f32r, full PE rate) need no big transposes beyond x itself
(PE-transposed per chunk).
"""

import sys

sys.path.insert(0, "/opt/trn_rl_repo")

import numpy as np
import concourse.bacc as bacc
import concourse.tile as tile
from concourse import mybir
from concourse.bass_utils import run_bass_kernel_spmd
from concourse.masks import make_identity

F32 = mybir.dt.float32
F32R = mybir.dt.float32r
BF16 = mybir.dt.bfloat16
MUL = mybir.AluOpType.mult
ADD = mybir.AluOpType.add

B, N, D = 8, 4096, 1024
H, HD = 16, 64
P = 128
NT = D // P          # 8 col-partition tiles
CH = 512             # n-chunk
NCH = N // CH        # 8 chunks
MS = CH // P         # 4 n-subtiles per chunk


def build():
    nc = bacc.Bacc()
    x_t = nc.dram_tensor("x", [N, D], F32, kind="ExternalInput")
    wq_t = nc.dram_tensor("wqT", [D, D], F32R, kind="ExternalInput")    # W_qkv.T
    wo_t = nc.dram_tensor("woT", [D, D], F32R, kind="ExternalInput")    # W_out.T
    temp_t = nc.dram_tensor("temp", [H, 1], F32, kind="ExternalInput")
    sel_t = nc.dram_tensor("sel", [NT, H, P], F32, kind="ExternalInput")
    selr_t = nc.dram_tensor("selr", [NT, H, P], F32R, kind="ExternalInput")
    bias_t = nc.dram_tensor("bout", [1, D], F32R, kind="ExternalInput")
    y_t = nc.dram_tensor("y", [N, D], F32, kind="ExternalOutput")

    with tile.TileContext(nc) as tc:
        with (
            tc.tile_pool(name="consts", bufs=1) as consts,
            tc.tile_pool(name="wmat", bufs=1) as wmat,
            tc.tile_pool(name="wsb", bufs=1) as wsb,
            tc.tile_pool(name="small", bufs=1) as small,
        ):
            # ---------- constants ----------
            ident = consts.tile([P, P], F32)
            make_identity(nc, ident)
            temp_sb = consts.tile([H, 1], F32)
            nc.sync.dma_start(out=temp_sb, in_=temp_t[:, :])
            bias_r = consts.tile([1, D], F32R)
            nc.sync.dma_start(out=bias_r, in_=bias_t[:, :])
            ones16 = consts.tile([H, 1], F32)
            nc.vector.memset(ones16, 1.0)
            ones1x16 = consts.tile([1, H], F32)
            nc.vector.memset(ones1x16, 1.0)
            ones1x128 = consts.tile([1, P], F32)
            nc.vector.memset(ones1x128, 1.0)
            ones1x128_r = consts.tile([1, P], F32R)
            nc.vector.tensor_copy(ones1x128_r, ones1x128)
            ones1x16r = consts.tile([1, H], F32R)
            nc.vector.tensor_copy(ones1x16r, ones1x16)

            # per-tile selectors (host constant): Sel01[t][j, p] = 1 iff j == 2t + p//64
            sel_f32 = []
            sel_r = []
            for t in range(NT):
                sf = consts.tile([H, P], F32, tag=f"self{t}", name=f"self{t}")
                nc.sync.dma_start(out=sf, in_=sel_t[t, :, :])
                sr = consts.tile([H, P], F32R, tag=f"selr{t}", name=f"selr{t}")
                nc.sync.dma_start(out=sr, in_=selr_t[t, :, :])
                sel_f32.append(sf)
                sel_r.append(sr)

            # weights (host pre-transposed); wq and wo share one slot (bufs=1):
            # wo is DMA'd after phase A frees wq
            wq_sb = wmat.tile([P, NT, D], F32R, tag="wm")
            for k in range(NT):
                nc.sync.dma_start(out=wq_sb[:, k, :], in_=wq_t[k * P : (k + 1) * P, :])

            # persistent big tensors
            w_tiles = [wsb.tile([P, N], BF16, tag=f"w{t}", name=f"w{t}") for t in range(NT)]
            s_strip = [small.tile([P, NCH], F32, tag=f"ss{t}", name=f"ss{t}") for t in range(NT)]
            d_strip = [small.tile([P, NCH], F32, tag=f"ds{t}", name=f"ds{t}") for t in range(NT)]

            # ---------- phase A: w = x @ WqkvT, s = sum_n w^2 ----------
            with (
                tc.tile_pool(name="achunk", bufs=2) as achunk,
                tc.tile_pool(name="scrA", bufs=3) as scrA,
                tc.tile_pool(name="psA", bufs=2, space="PSUM") as psA,
                tc.tile_pool(name="psTP", bufs=2, space="PSUM") as psTP,
            ):
                for c in range(NCH):
                    cs = slice(c * CH, (c + 1) * CH)
                    x_raw = achunk.tile([P, MS, D], F32, tag="xraw")
                    nc.sync.dma_start(
                        out=x_raw,
                        in_=x_t[cs, :].rearrange("(m p) i -> p m i", p=P),
                    )
                    xT = achunk.tile([P, NT, CH], F32R, tag="xT", bufs=1)
                    for k in range(NT):
                        tp_ps = psTP.tile([P, CH], F32, tag="tp")
                        for m in range(MS):
                            nc.tensor.transpose(
                                tp_ps[:, m * P : (m + 1) * P],
                                x_raw[:, m, k * P : (k + 1) * P],
                                ident,
                            )
                        nc.scalar.copy(out=xT[:, k, :], in_=tp_ps)
                    for t in range(NT):
                        w_ps = psA.tile([P, CH], F32, tag="mm1")
                        for k in range(NT):
                            nc.tensor.matmul(
                                w_ps,
                                wq_sb[:, k, t * P : (t + 1) * P],
                                xT[:, k, :],
                                start=(k == 0),
                                stop=(k == NT - 1),
                            )
                        nc.scalar.copy(out=w_tiles[t][:, cs], in_=w_ps)
                        junk = scrA.tile([P, CH], BF16, tag="junkA")
                        nc.vector.scalar_tensor_tensor(
                            out=junk,
                            in0=w_tiles[t][:, cs],
                            scalar=1.0,
                            in1=w_tiles[t][:, cs],
                            op0=MUL,
                            op1=MUL,
                            accum_out=s_strip[t][:, c : c + 1],
                        )

            # W_out.T load (reuses wq's slot; overlaps softmax phases)
            wo_sb = wmat.tile([P, NT, D], F32R, tag="wm")
            for k in range(NT):
                nc.sync.dma_start(out=wo_sb[:, k, :], in_=wo_t[k * P : (k + 1) * P, :])

            # softmax-side pool opens only after phase A frees its space
            soft = tc.alloc_tile_pool(name="soft", bufs=1)

            # ---------- stats 1: inv_temp, L_big ----------
            lbig = []
            with tc.tile_pool(name="psS1", bufs=2, space="PSUM") as psS1:
                for t in range(NT):
                    s_all = small.tile([P, 1], F32, tag=f"sall{t}")
                    nc.vector.reduce_sum(s_all, s_strip[t], axis=mybir.AxisListType.X)
                    nc.vector.tensor_scalar_max(out=s_all, in0=s_all, scalar1=1e-24)
                    rcp = small.tile([P, 1], F32, tag=f"rcp{t}")
                    nc.vector.reciprocal(rcp, s_all)
                    tb_ps = psS1.tile([P, 1], F32, tag="tb")
                    nc.tensor.matmul(tb_ps, sel_f32[t], temp_sb, start=True, stop=True)
                    inv_t = small.tile([P, 1], F32, tag=f"invt{t}")
                    nc.vector.tensor_mul(inv_t, rcp, tb_ps)
                    lb = small.tile([P, H], BF16, tag=f"lbig{t}")
                    nc.vector.memset(lb, 0.0)
                    nc.vector.tensor_copy(lb[0:HD, 2 * t : 2 * t + 1], inv_t[0:HD, :])
                    nc.vector.tensor_copy(
                        lb[HD:P, 2 * t + 1 : 2 * t + 2], inv_t[HD:P, :]
                    )
                    lbig.append(lb)

            # ---------- phase L: logits[h, n] (squares on DVE, copies on ACT) ----------
            logits = soft.tile([H, N], F32R, tag="logits")
            with (
                tc.tile_pool(name="scrL", bufs=3) as scrL,
                tc.tile_pool(name="psL", bufs=2, space="PSUM") as psL,
            ):
                for c in range(NCH):
                    cs = slice(c * CH, (c + 1) * CH)
                    lg_ps = psL.tile([H, CH], F32, tag="lg")
                    for t in range(NT):
                        w2t = scrL.tile([P, CH], BF16, tag="w2t")
                        nc.vector.tensor_mul(
                            w2t, w_tiles[t][:, cs], w_tiles[t][:, cs]
                        )
                        nc.tensor.matmul(
                            lg_ps, lbig[t], w2t, start=(t == 0), stop=(t == NT - 1)
                        )
                    nc.scalar.copy(out=logits[:, cs], in_=lg_ps)

            # ---------- softmax over h, log-sum-exp form ----------
            # Pi = exp(logits - ln(sum_h exp(logits))); avoids a 4096-wide
            # iterative reciprocal on DVE.
            epool = tc.alloc_tile_pool(name="epool", bufs=1)
            e_hn = epool.tile([H, N], BF16, tag="ehn")
            nc.scalar.activation(
                out=e_hn, in_=logits.bitcast(F32),
                func=mybir.ActivationFunctionType.Exp,
            )
            ones16b = consts.tile([H, 1], BF16)
            nc.vector.tensor_copy(ones16b, ones16)
            # Pi overwrites logits in place (exp(logits - lnS) reads+writes same tile)
            pi_hn = logits
            sume_row = small.tile([1, N], F32, tag="sumerow")
            lns_row = small.tile([1, N], F32R, tag="lnsrow")
            with tc.tile_pool(name="psSM", bufs=2, space="PSUM") as psSM:
                for c in range(NCH):
                    cs = slice(c * CH, (c + 1) * CH)
                    se_ps = psSM.tile([1, CH], F32, tag="se")
                    nc.tensor.matmul(
                        se_ps, ones16b, e_hn[:, cs], start=True, stop=True
                    )
                    nc.scalar.copy(out=sume_row[:, cs], in_=se_ps)
                nc.scalar.activation(
                    out=lns_row, in_=sume_row, func=mybir.ActivationFunctionType.Ln
                )
                for c in range(NCH):
                    cs = slice(c * CH, (c + 1) * CH)
                    lnb_ps = psSM.tile([H, CH], F32, tag="lnb")
                    nc.tensor.matmul(
                        lnb_ps, ones1x16r, lns_row[:, cs], start=True, stop=True
                    )
                    nc.vector.tensor_sub(
                        logits[:, cs], logits[:, cs].bitcast(F32), lnb_ps
                    )
                    nc.scalar.activation(
                        out=pi_hn[:, cs],
                        in_=logits[:, cs].bitcast(F32),
                        func=mybir.ActivationFunctionType.Exp,
                    )

            epool.release()

            sumpi = small.tile([H, 1], F32, tag="sumpi")
            nc.vector.reduce_sum(
                sumpi, pi_hn.bitcast(F32), axis=mybir.AxisListType.X
            )
            nc.vector.tensor_scalar_add(out=sumpi, in0=sumpi, scalar1=1e-8)
            ispi = small.tile([H, 1], F32, tag="ispi")
            nc.vector.reciprocal(ispi, sumpi)

            # ---------- phase B (merged): u = w*Pi_b overwrites w; dots ----------
            with (
                tc.tile_pool(name="scrB", bufs=3) as scrB,
                tc.tile_pool(name="psB1", bufs=2, space="PSUM") as psB1,
            ):
                for c in range(NCH):
                    cs = slice(c * CH, (c + 1) * CH)
                    for t in range(NT):
                        pib_ps = psB1.tile([P, CH], F32, tag="pib")
                        nc.tensor.matmul(
                            pib_ps, sel_r[t], pi_hn[:, cs], start=True, stop=True
                        )
                        u_tmp = scrB.tile([P, CH], BF16, tag="utmp")
                        nc.vector.tensor_mul(u_tmp, w_tiles[t][:, cs], pib_ps)
                        junk = scrB.tile([P, CH], BF16, tag="junkB")
                        nc.vector.scalar_tensor_tensor(
                            out=junk,
                            in0=u_tmp,
                            scalar=1.0,
                            in1=w_tiles[t][:, cs],
                            op0=MUL,
                            op1=MUL,
                            accum_out=d_strip[t][:, c : c + 1],
                        )
                        # u overwrites w in place (WAR on the STT above)
                        nc.gpsimd.tensor_copy(out=w_tiles[t][:, cs], in_=u_tmp)

            # ---------- stats 2: attn; W' = -attn * WoutT (bf16) ----------
            wob, _wob_free = tc.tile([P, NT, D], BF16, name="wob")
            bias_bf = consts.tile([1, D], BF16)
            nc.vector.tensor_copy(bias_bf, bias_r.bitcast(F32))
            ones1x128b = consts.tile([1, P], BF16)
            nc.vector.memset(ones1x128b, 1.0)
            with tc.tile_pool(name="psS2", bufs=2, space="PSUM") as psS2:
                for t in range(NT):
                    isp_ps = psS2.tile([P, 1], F32, tag="isp")
                    nc.tensor.matmul(isp_ps, sel_f32[t], ispi, start=True, stop=True)
                    dots = small.tile([P, 1], F32, tag=f"dots{t}")
                    nc.vector.reduce_sum(dots, d_strip[t], axis=mybir.AxisListType.X)
                    nc.vector.tensor_mul(dots, dots, isp_ps)
                    nc.vector.tensor_scalar_add(out=dots, in0=dots, scalar1=1.0)
                    attn = small.tile([P, 1], F32, tag=f"attn{t}")
                    nc.vector.reciprocal(attn, dots)
                    nc.vector.tensor_scalar_mul(out=attn, in0=attn, scalar1=-1.0)
                    nc.vector.tensor_scalar_mul(
                        out=wob[:, t, :],
                        in0=wo_sb[:, t, :].bitcast(F32),
                        scalar1=attn,
                    )

            # ---------- phase MM2: y = u.T @ W' + b (dense PE) ----------
            with (
                tc.tile_pool(name="och", bufs=1) as och,
                tc.tile_pool(name="psMM2", bufs=4, space="PSUM") as psMM2,
            ):
                for c in range(NCH):
                    cs = slice(c * CH, (c + 1) * CH)
                    outf = och.tile([P, MS, D], F32, tag="outf")
                    for m in range(MS):
                        ms_ = slice(c * CH + m * P, c * CH + (m + 1) * P)
                        for oh in range(2):
                            os_ = slice(oh * CH, (oh + 1) * CH)
                            f_ps = psMM2.tile([P, CH], F32, tag="mm2")
                            for t in range(NT):
                                nc.tensor.matmul(
                                    f_ps,
                                    w_tiles[t][:, ms_],
                                    wob[:, t, os_],
                                    start=(t == 0),
                                    stop=False,
                                )
                            nc.tensor.matmul(
                                f_ps,
                                ones1x128b,
                                bias_bf[:, os_],
                                start=False,
                                stop=True,
                            )
                            nc.scalar.copy(out=outf[:, m, os_], in_=f_ps)
                    nc.sync.dma_start(
                        out=y_t[cs, :].rearrange("(m p) i -> p m i", p=P),
                        in_=outf,
                    )
            _wob_free()
            soft.release()

    if not nc.is_finalized():
        nc.finalize()
    return nc


_NC_CACHE = None
_LAST_IN_MAPS = None
_RUNNER = None


def _make_runner(nc, n_cores):
    """Like bass2jax.run_bass_via_pjrt but with the jitted callable cached,
    so repeat calls don't re-trace/re-compile the XLA wrapper."""
    import jax
    from jax.experimental.shard_map import shard_map
    from jax.sharding import Mesh, PartitionSpec
    from concourse import mybir as _mybir
    from concourse.bass2jax import (
        _bass_exec_p,
        install_neuronx_cc_hook,
        partition_id_tensor,
    )

    install_neuronx_cc_hook()

    partition_name = nc.partition_id_tensor.name if nc.partition_id_tensor else None
    in_names, out_names, out_avals, zero_outs = [], [], [], []
    for alloc in nc.m.functions[0].allocations:
        if not isinstance(alloc, _mybir.MemoryLocationSet):
            continue
        name = alloc.memorylocations[0].name
        if alloc.kind == "ExternalInput":
            if name != partition_name:
                in_names.append(name)
        elif alloc.kind == "ExternalOutput":
            shape = tuple(alloc.tensor_shape)
            dtype = _mybir.dt.np(alloc.dtype)
            out_names.append(name)
            out_avals.append(jax.core.ShapedArray(shape, dtype))
            zero_outs.append(np.zeros(shape, dtype))
    n_params = len(in_names)
    n_outs = len(out_names)
    all_in_names = in_names + out_names + (
        [partition_name] if partition_name else []
    )
    donate = tuple(range(n_params, n_params + n_outs))

    def _body(*args):
        operands = list(args)
        if partition_name is not None:
            operands.append(partition_id_tensor())
        outs = _bass_exec_p.bind(
            *operands,
            out_avals=tuple(out_avals),
            in_names=tuple(all_in_names),
            out_names=tuple(out_names),
            lowering_input_output_aliases=(),
            sim_require_finite=True,
            sim_require_nnan=True,
            nc=nc,
        )
        return tuple(outs)

    devices = jax.devices()[:n_cores]
    mesh = Mesh(np.asarray(devices), ("core",))
    in_specs = (PartitionSpec("core"),) * (n_params + n_outs)
    out_specs = (PartitionSpec("core"),) * n_outs
    sharded = jax.jit(
        shard_map(
            _body, mesh=mesh, in_specs=in_specs, out_specs=out_specs, check_rep=False
        ),
        donate_argnums=donate,
        keep_unused=True,
    )

    def run(in_maps):
        concat_in = [
            np.concatenate([np.asarray(m[name]) for m in in_maps], axis=0)
            for name in in_names
        ]
        concat_zeros = [
            np.zeros((n_cores * z.shape[0], *z.shape[1:]), z.dtype)
            for z in zero_outs
        ]
        out_arrs = sharded(*concat_in, *concat_zeros)
        return {
            name: np.asarray(out_arrs[i]).reshape(n_cores, *out_avals[i].shape)
            for i, name in enumerate(out_names)
        }

    run.sharded = sharded
    run.meta = (in_names, out_names, out_avals, n_params, n_outs)
    return run


def kernel(x, W_qkv, temp, W_out, b_out):
    global _NC_CACHE, _RUNNER
    if _NC_CACHE is None:
        _NC_CACHE = build()
        _RUNNER = _make_runner(_NC_CACHE, B)
    nc = _NC_CACHE

    x = np.asarray(x, dtype=np.float32)
    wqT = np.ascontiguousarray(np.asarray(W_qkv, dtype=np.float32).T)
    woT = np.ascontiguousarray(np.asarray(W_out, dtype=np.float32).T)
    temp = np.ascontiguousarray(np.asarray(temp, dtype=np.float32).reshape(H, 1))
    bout = np.ascontiguousarray(np.asarray(b_out, dtype=np.float32).reshape(1, D))

    sel = np.zeros((NT, H, P), dtype=np.float32)
    for t in range(NT):
        sel[t, 2 * t, 0:HD] = 1.0
        sel[t, 2 * t + 1, HD:P] = 1.0

    core_ids = list(range(B))
    in_maps = [
        {"x": np.ascontiguousarray(x[i]), "wqT": wqT, "woT": woT,
         "temp": temp, "bout": bout, "sel": sel, "selr": sel}
        for i in core_ids
    ]
    global _LAST_IN_MAPS
    _LAST_IN_MAPS = in_maps
    out = _RUNNER(in_maps)
    return out["y"]


if __name__ == "__main__":
    rng = np.random.default_rng(0)
    x = rng.standard_normal((B, N, D), dtype=np.float32)
    W_qkv = (rng.standard_normal((D, D), dtype=np.float32) * 0.02).astype(np.float32)
    temp = np.ones((H, 1), dtype=np.float32)
    W_out = (rng.standard_normal((D, D), dtype=np.float32) * 0.02).astype(np.float32)
    b_out = np.zeros((D,), dtype=np.float32)
    y = kernel(x=x, W_qkv=W_qkv, temp=temp, W_out=W_out, b_out=b_out)
    print("kernel ran, y shape", y.shape, "mean abs", np.abs(y).mean())


# revision 21
# speedup vs baseline: 9792.4559x; 9094.0311x over previous
"""AttentionTSSA Trainium2 kernel.

Sharding: data-parallel over batch. B=8 -> one batch element per NeuronCore,
zero collectives. Host slices inputs / stacks outputs.

Per-core math (x: [N=4096, D=1024], heads h=16, head dim d=64):
  w[n, c]   = x @ W_qkv.T                   (c = hd flattened head*64+dd)
  s[c]      = sum_n w^2                     (col norms squared)
  logits[h,n] = temp[h] * sum_dd w^2[hd,n] / max(s[hd], 1e-24)
  Pi        = softmax_h(logits)
  dots[c]   = (sum_n Pi[h,n] * w^2[c,n]) / (sum_n Pi[h,n] + 1e-8)
  attn[c]   = 1 / (1 + dots)
  y         = (-(w * Pi_bcast) * attn_bcast) @ W_out.T + b_out

On-chip layout: w stored column-major [c(part), n(free)] as 8 tiles
[128, 4096] bf16, so every sum_n is a free-axis reduce and both big
matmuls (f32r, full PE rate) need no big transposes beyond x itself
(PE-transposed per chunk).
"""

import sys

sys.path.insert(0, "/opt/trn_rl_repo")

import numpy as np
import concourse.bacc as bacc
import concourse.tile as tile
from concourse import mybir
from concourse.bass_utils import run_bass_kernel_spmd
from concourse.masks import make_identity

F32 = mybir.dt.float32
F32R = mybir.dt.float32r
BF16 = mybir.dt.bfloat16
MUL = mybir.AluOpType.mult
ADD = mybir.AluOpType.add

B, N, D = 8, 4096, 1024
H, HD = 16, 64
P = 128
NT = D // P          # 8 col-partition tiles
CH = 512             # n-chunk
NCH = N // CH        # 8 chunks
MS = CH // P         # 4 n-subtiles per chunk


def build():
    nc = bacc.Bacc()
    x_t = nc.dram_tensor("x", [N, D], F32, kind="ExternalInput")
    wq_t = nc.dram_tensor("wqT", [D, D], F32R, kind="ExternalInput")    # W_qkv.T
    wo_t = nc.dram_tensor("woT", [D, D], F32R, kind="ExternalInput")    # W_out.T
    temp_t = nc.dram_tensor("temp", [H, 1], F32, kind="ExternalInput")
    sel_t = nc.dram_tensor("sel", [NT, H, P], F32, kind="ExternalInput")
    selr_t = nc.dram_tensor("selr", [NT, H, P], F32R, kind="ExternalInput")
    bias_t = nc.dram_tensor("bout", [1, D], F32R, kind="ExternalInput")
    y_t = nc.dram_tensor("y", [N, D], F32, kind="ExternalOutput")

    with tile.TileContext(nc) as tc:
        with (
            tc.tile_pool(name="consts", bufs=1) as consts,
            tc.tile_pool(name="wmat", bufs=1) as wmat,
            tc.tile_pool(name="wsb", bufs=1) as wsb,
            tc.tile_pool(name="small", bufs=1) as small,
        ):
            # ---------- constants ----------
            ident = consts.tile([P, P], F32)
            make_identity(nc, ident)
            temp_sb = consts.tile([H, 1], F32)
            nc.sync.dma_start(out=temp_sb, in_=temp_t[:, :])
            bias_r = consts.tile([1, D], F32R)
            nc.sync.dma_start(out=bias_r, in_=bias_t[:, :])
            ones16 = consts.tile([H, 1], F32)
            nc.vector.memset(ones16, 1.0)
            ones1x16 = consts.tile([1, H], F32)
            nc.vector.memset(ones1x16, 1.0)
            ones1x128 = consts.tile([1, P], F32)
            nc.vector.memset(ones1x128, 1.0)
            ones1x128_r = consts.tile([1, P], F32R)
            nc.vector.tensor_copy(ones1x128_r, ones1x128)
            ones1x16r = consts.tile([1, H], F32R)
            nc.vector.tensor_copy(ones1x16r, ones1x16)

            # per-tile selectors (host constant): Sel01[t][j, p] = 1 iff j == 2t + p//64
            sel_f32 = []
            sel_r = []
            for t in range(NT):
                sf = consts.tile([H, P], F32, tag=f"self{t}", name=f"self{t}")
                nc.sync.dma_start(out=sf, in_=sel_t[t, :, :])
                sr = consts.tile([H, P], F32R, tag=f"selr{t}", name=f"selr{t}")
                nc.sync.dma_start(out=sr, in_=selr_t[t, :, :])
                sel_f32.append(sf)
                sel_r.append(sr)

            # weights (host pre-transposed); wq and wo share one slot (bufs=1):
            # wo is DMA'd after phase A frees wq
            wq_sb = wmat.tile([P, NT, D], F32R, tag="wm")
            for k in range(NT):
                nc.sync.dma_start(out=wq_sb[:, k, :], in_=wq_t[k * P : (k + 1) * P, :])

            # persistent big tensors
            w_tiles = [wsb.tile([P, N], BF16, tag=f"w{t}", name=f"w{t}") for t in range(NT)]
            s_strip = [small.tile([P, NCH], F32, tag=f"ss{t}", name=f"ss{t}") for t in range(NT)]
            d_strip = [small.tile([P, NCH], F32, tag=f"ds{t}", name=f"ds{t}") for t in range(NT)]

            # ---------- phase A: w = x @ WqkvT, s = sum_n w^2 ----------
            with (
                tc.tile_pool(name="achunk", bufs=2) as achunk,
                tc.tile_pool(name="scrA", bufs=3) as scrA,
                tc.tile_pool(name="psA", bufs=2, space="PSUM") as psA,
                tc.tile_pool(name="psTP", bufs=2, space="PSUM") as psTP,
            ):
                for c in range(NCH):
                    cs = slice(c * CH, (c + 1) * CH)
                    x_raw = achunk.tile([P, MS, D], F32, tag="xraw")
                    nc.sync.dma_start(
                        out=x_raw,
                        in_=x_t[cs, :].rearrange("(m p) i -> p m i", p=P),
                    )
                    xT = achunk.tile([P, NT, CH], F32R, tag="xT", bufs=1)
                    for k in range(NT):
                        tp_ps = psTP.tile([P, CH], F32, tag="tp")
                        for m in range(MS):
                            nc.tensor.transpose(
                                tp_ps[:, m * P : (m + 1) * P],
                                x_raw[:, m, k * P : (k + 1) * P],
                                ident,
                            )
                        nc.scalar.copy(out=xT[:, k, :], in_=tp_ps)
                    for t in range(NT):
                        w_ps = psA.tile([P, CH], F32, tag="mm1")
                        for k in range(NT):
                            nc.tensor.matmul(
                                w_ps,
                                wq_sb[:, k, t * P : (t + 1) * P],
                                xT[:, k, :],
                                start=(k == 0),
                                stop=(k == NT - 1),
                            )
                        nc.scalar.copy(out=w_tiles[t][:, cs], in_=w_ps)
                        junk = scrA.tile([P, CH], BF16, tag="junkA")
                        nc.vector.scalar_tensor_tensor(
                            out=junk,
                            in0=w_tiles[t][:, cs],
                            scalar=1.0,
                            in1=w_tiles[t][:, cs],
                            op0=MUL,
                            op1=MUL,
                            accum_out=s_strip[t][:, c : c + 1],
                        )

            # W_out.T load (reuses wq's slot; overlaps softmax phases)
            wo_sb = wmat.tile([P, NT, D], F32R, tag="wm")
            for k in range(NT):
                nc.sync.dma_start(out=wo_sb[:, k, :], in_=wo_t[k * P : (k + 1) * P, :])

            # softmax-side pool opens only after phase A frees its space
            soft = tc.alloc_tile_pool(name="soft", bufs=1)

            # ---------- stats 1: inv_temp, L_big ----------
            lbig = []
            with tc.tile_pool(name="psS1", bufs=2, space="PSUM") as psS1:
                for t in range(NT):
                    s_all = small.tile([P, 1], F32, tag=f"sall{t}")
                    nc.vector.reduce_sum(s_all, s_strip[t], axis=mybir.AxisListType.X)
                    nc.vector.tensor_scalar_max(out=s_all, in0=s_all, scalar1=1e-24)
                    rcp = small.tile([P, 1], F32, tag=f"rcp{t}")
                    nc.vector.reciprocal(rcp, s_all)
                    tb_ps = psS1.tile([P, 1], F32, tag="tb")
                    nc.tensor.matmul(tb_ps, sel_f32[t], temp_sb, start=True, stop=True)
                    inv_t = small.tile([P, 1], F32, tag=f"invt{t}")
                    nc.vector.tensor_mul(inv_t, rcp, tb_ps)
                    lb = small.tile([P, H], BF16, tag=f"lbig{t}")
                    nc.vector.memset(lb, 0.0)
                    nc.vector.tensor_copy(lb[0:HD, 2 * t : 2 * t + 1], inv_t[0:HD, :])
                    nc.vector.tensor_copy(
                        lb[HD:P, 2 * t + 1 : 2 * t + 2], inv_t[HD:P, :]
                    )
                    lbig.append(lb)

            # ---------- phase L: logits[h, n] (squares on DVE, copies on ACT) ----------
            logits = soft.tile([H, N], F32R, tag="logits")
            with (
                tc.tile_pool(name="scrL", bufs=3) as scrL,
                tc.tile_pool(name="psL", bufs=2, space="PSUM") as psL,
            ):
                for c in range(NCH):
                    cs = slice(c * CH, (c + 1) * CH)
                    lg_ps = psL.tile([H, CH], F32, tag="lg")
                    for t in range(NT):
                        w2t = scrL.tile([P, CH], BF16, tag="w2t")
                        nc.vector.tensor_mul(
                            w2t, w_tiles[t][:, cs], w_tiles[t][:, cs]
                        )
                        nc.tensor.matmul(
                            lg_ps, lbig[t], w2t, start=(t == 0), stop=(t == NT - 1)
                        )
                    nc.scalar.copy(out=logits[:, cs], in_=lg_ps)

            # ---------- softmax over h, log-sum-exp form ----------
            # Pi = exp(logits - ln(sum_h exp(logits))); avoids a 4096-wide
            # iterative reciprocal on DVE.
            epool = tc.alloc_tile_pool(name="epool", bufs=1)
            e_hn = epool.tile([H, N], F32R, tag="ehn")
            nc.scalar.activation(
                out=e_hn, in_=logits.bitcast(F32),
                func=mybir.ActivationFunctionType.Exp,
            )
            ones16r = consts.tile([H, 1], F32R)
            nc.vector.tensor_copy(ones16r, ones16)
            # Pi overwrites logits in place (exp(logits - lnS) reads+writes same tile)
            pi_hn = logits
            sume_row = small.tile([1, N], F32, tag="sumerow")
            lns_row = small.tile([1, N], F32R, tag="lnsrow")
            with tc.tile_pool(name="psSM", bufs=2, space="PSUM") as psSM:
                for c in range(NCH):
                    cs = slice(c * CH, (c + 1) * CH)
                    se_ps = psSM.tile([1, CH], F32, tag="se")
                    nc.tensor.matmul(
                        se_ps, ones16r, e_hn[:, cs], start=True, stop=True
                    )
                    nc.scalar.copy(out=sume_row[:, cs], in_=se_ps)
                nc.scalar.activation(
                    out=lns_row, in_=sume_row, func=mybir.ActivationFunctionType.Ln
                )
                for c in range(NCH):
                    cs = slice(c * CH, (c + 1) * CH)
                    lnb_ps = psSM.tile([H, CH], F32, tag="lnb")
                    nc.tensor.matmul(
                        lnb_ps, ones1x16r, lns_row[:, cs], start=True, stop=True
                    )
                    nc.vector.tensor_sub(
                        logits[:, cs], logits[:, cs].bitcast(F32), lnb_ps
                    )
                    nc.scalar.activation(
                        out=pi_hn[:, cs],
                        in_=logits[:, cs].bitcast(F32),
                        func=mybir.ActivationFunctionType.Exp,
                    )

            epool.release()

            sumpi = small.tile([H, 1], F32, tag="sumpi")
            nc.vector.reduce_sum(
                sumpi, pi_hn.bitcast(F32), axis=mybir.AxisListType.X
            )
            nc.vector.tensor_scalar_add(out=sumpi, in0=sumpi, scalar1=1e-8)
            ispi = small.tile([H, 1], F32, tag="ispi")
            nc.vector.reciprocal(ispi, sumpi)

            # ---------- phase B (merged): u = w*Pi_b overwrites w; dots ----------
            # 1024-wide chunks to amortize fixed per-op costs
            CHB = 1024
            NCHB = N // CHB
            with (
                tc.tile_pool(name="scrB", bufs=3) as scrB,
                tc.tile_pool(name="psB1", bufs=2, space="PSUM") as psB1,
            ):
                for c in range(NCHB):
                    cs = slice(c * CHB, (c + 1) * CHB)
                    for t in range(NT):
                        pib_ps = psB1.tile([P, CHB], F32, tag="pib")
                        for hh in range(CHB // CH):
                            nc.tensor.matmul(
                                pib_ps[:, hh * CH : (hh + 1) * CH],
                                sel_r[t],
                                pi_hn[:, c * CHB + hh * CH : c * CHB + (hh + 1) * CH],
                                start=True,
                                stop=True,
                            )
                        pib_sb = scrB.tile([P, CHB], BF16, tag="pibsb")
                        nc.scalar.copy(out=pib_sb, in_=pib_ps)
                        u_tmp = scrB.tile([P, CHB], BF16, tag="utmp")
                        nc.vector.tensor_mul(u_tmp, w_tiles[t][:, cs], pib_sb)
                        junk = scrB.tile([P, CHB], BF16, tag="junkB")
                        nc.vector.scalar_tensor_tensor(
                            out=junk,
                            in0=u_tmp,
                            scalar=1.0,
                            in1=w_tiles[t][:, cs],
                            op0=MUL,
                            op1=MUL,
                            accum_out=d_strip[t][:, c : c + 1],
                        )
                        # u overwrites w in place (WAR on the STT above)
                        nc.gpsimd.tensor_copy(out=w_tiles[t][:, cs], in_=u_tmp)

            # ---------- stats 2: attn; W' = -attn * WoutT (bf16) ----------
            wob, _wob_free = tc.tile([P, NT, D], BF16, name="wob")
            bias_bf = consts.tile([1, D], BF16)
            nc.vector.tensor_copy(bias_bf, bias_r.bitcast(F32))
            ones1x128b = consts.tile([1, P], BF16)
            nc.vector.memset(ones1x128b, 1.0)
            with tc.tile_pool(name="psS2", bufs=2, space="PSUM") as psS2:
                for t in range(NT):
                    isp_ps = psS2.tile([P, 1], F32, tag="isp")
                    nc.tensor.matmul(isp_ps, sel_f32[t], ispi, start=True, stop=True)
                    dots = small.tile([P, 1], F32, tag=f"dots{t}")
                    nc.vector.reduce_sum(
                        dots, d_strip[t][:, 0 : N // 1024], axis=mybir.AxisListType.X
                    )
                    nc.vector.tensor_mul(dots, dots, isp_ps)
                    nc.vector.tensor_scalar_add(out=dots, in0=dots, scalar1=1.0)
                    attn = small.tile([P, 1], F32, tag=f"attn{t}")
                    nc.vector.reciprocal(attn, dots)
                    nc.vector.tensor_scalar_mul(out=attn, in0=attn, scalar1=-1.0)
                    nc.vector.tensor_scalar_mul(
                        out=wob[:, t, :],
                        in0=wo_sb[:, t, :].bitcast(F32),
                        scalar1=attn,
                    )

            # ---------- phase MM2: y = u.T @ W' + b (dense PE) ----------
            with (
                tc.tile_pool(name="och", bufs=1) as och,
                tc.tile_pool(name="psMM2", bufs=4, space="PSUM") as psMM2,
            ):
                for c in range(NCH):
                    cs = slice(c * CH, (c + 1) * CH)
                    outf = och.tile([P, MS, D], F32, tag="outf")
                    for m in range(MS):
                        ms_ = slice(c * CH + m * P, c * CH + (m + 1) * P)
                        for oh in range(2):
                            os_ = slice(oh * CH, (oh + 1) * CH)
                            f_ps = psMM2.tile([P, CH], F32, tag="mm2")
                            for t in range(NT):
                                nc.tensor.matmul(
                                    f_ps,
                                    w_tiles[t][:, ms_],
                                    wob[:, t, os_],
                                    start=(t == 0),
                                    stop=False,
                                )
                            nc.tensor.matmul(
                                f_ps,
                                ones1x128b,
                                bias_bf[:, os_],
                                start=False,
                                stop=True,
                            )
                            nc.scalar.copy(out=outf[:, m, os_], in_=f_ps)
                    nc.sync.dma_start(
                        out=y_t[cs, :].rearrange("(m p) i -> p m i", p=P),
                        in_=outf,
                    )
            _wob_free()
            soft.release()

    if not nc.is_finalized():
        nc.finalize()
    return nc


_NC_CACHE = None
_LAST_IN_MAPS = None
_RUNNER = None


def _make_runner(nc, n_cores):
    """Like bass2jax.run_bass_via_pjrt but with the jitted callable cached,
    so repeat calls don't re-trace/re-compile the XLA wrapper."""
    import jax
    from jax.experimental.shard_map import shard_map
    from jax.sharding import Mesh, PartitionSpec
    from concourse import mybir as _mybir
    from concourse.bass2jax import (
        _bass_exec_p,
        install_neuronx_cc_hook,
        partition_id_tensor,
    )

    install_neuronx_cc_hook()

    partition_name = nc.partition_id_tensor.name if nc.partition_id_tensor else None
    in_names, out_names, out_avals, zero_outs = [], [], [], []
    for alloc in nc.m.functions[0].allocations:
        if not isinstance(alloc, _mybir.MemoryLocationSet):
            continue
        name = alloc.memorylocations[0].name
        if alloc.kind == "ExternalInput":
            if name != partition_name:
                in_names.append(name)
        elif alloc.kind == "ExternalOutput":
            shape = tuple(alloc.tensor_shape)
            dtype = _mybir.dt.np(alloc.dtype)
            out_names.append(name)
            out_avals.append(jax.core.ShapedArray(shape, dtype))
            zero_outs.append(np.zeros(shape, dtype))
    n_params = len(in_names)
    n_outs = len(out_names)
    all_in_names = in_names + out_names + (
        [partition_name] if partition_name else []
    )
    donate = tuple(range(n_params, n_params + n_outs))

    def _body(*args):
        operands = list(args)
        if partition_name is not None:
            operands.append(partition_id_tensor())
        outs = _bass_exec_p.bind(
            *operands,
            out_avals=tuple(out_avals),
            in_names=tuple(all_in_names),
            out_names=tuple(out_names),
            lowering_input_output_aliases=(),
            sim_require_finite=True,
            sim_require_nnan=True,
            nc=nc,
        )
        return tuple(outs)

    devices = jax.devices()[:n_cores]
    mesh = Mesh(np.asarray(devices), ("core",))
    in_specs = (PartitionSpec("core"),) * (n_params + n_outs)
    out_specs = (PartitionSpec("core"),) * n_outs
    sharded = jax.jit(
        shard_map(
            _body, mesh=mesh, in_specs=in_specs, out_specs=out_specs, check_rep=False
        ),
        donate_argnums=donate,
        keep_unused=True,
    )

    def run(in_maps):
        concat_in = [
            np.concatenate([np.asarray(m[name]) for m in in_maps], axis=0)
            for name in in_names
        ]
        concat_zeros = [
            np.zeros((n_cores * z.shape[0], *z.shape[1:]), z.dtype)
            for z in zero_outs
        ]
        out_arrs = sharded(*concat_in, *concat_zeros)
        return {
            name: np.asarray(out_arrs[i]).reshape(n_cores, *out_avals[i].shape)
            for i, name in enumerate(out_names)
        }

    run.sharded = sharded
    run.meta = (in_names, out_names, out_avals, n_params, n_outs)
    return run


def kernel(x, W_qkv, temp, W_out, b_out):
    global _NC_CACHE, _RUNNER
    if _NC_CACHE is None:
        _NC_CACHE = build()
        _RUNNER = _make_runner(_NC_CACHE, B)
    nc = _NC_CACHE

    x = np.asarray(x, dtype=np.float32)
    wqT = np.ascontiguousarray(np.asarray(W_qkv, dtype=np.float32).T)
    woT = np.ascontiguousarray(np.asarray(W_out, dtype=np.float32).T)
    temp = np.ascontiguousarray(np.asarray(temp, dtype=np.float32).reshape(H, 1))
    bout = np.ascontiguousarray(np.asarray(b_out, dtype=np.float32).reshape(1, D))

    sel = np.zeros((NT, H, P), dtype=np.float32)
    for t in range(NT):
        sel[t, 2 * t, 0:HD] = 1.0
        sel[t, 2 * t + 1, HD:P] = 1.0

    core_ids = list(range(B))
    in_maps = [
        {"x": np.ascontiguousarray(x[i]), "wqT": wqT, "woT": woT,
         "temp": temp, "bout": bout, "sel": sel, "selr": sel}
        for i in core_ids
    ]
    global _LAST_IN_MAPS
    _LAST_IN_MAPS = in_maps
    out = _RUNNER(in_maps)
    return out["y"]


if __name__ == "__main__":
    rng = np.random.default_rng(0)
    x = rng.standard_normal((B, N, D), dtype=np.float32)
    W_qkv = (rng.standard_normal((D, D), dtype=np.float32) * 0.02).astype(np.float32)
    temp = np.ones((H, 1), dtype=np.float32)
    W_out = (rng.standard_normal((D, D), dtype=np.float32) * 0.02).astype(np.float32)
    b_out = np.zeros((D,), dtype=np.float32)
    y = kernel(x=x, W_qkv=W_qkv, temp=temp, W_out=W_out, b_out=b_out)
    print("kernel ran, y shape", y.shape, "mean abs", np.abs(y).mean())


# revision 22
# speedup vs baseline: 13869.5294x; 1.4163x over previous
"""AttentionTSSA Trainium2 kernel.

Sharding: data-parallel over batch. B=8 -> one batch element per NeuronCore,
zero collectives. Host slices inputs / stacks outputs.

Per-core math (x: [N=4096, D=1024], heads h=16, head dim d=64):
  w[n, c]   = x @ W_qkv.T                   (c = hd flattened head*64+dd)
  s[c]      = sum_n w^2                     (col norms squared)
  logits[h,n] = temp[h] * sum_dd w^2[hd,n] / max(s[hd], 1e-24)
  Pi        = softmax_h(logits)
  dots[c]   = (sum_n Pi[h,n] * w^2[c,n]) / (sum_n Pi[h,n] + 1e-8)
  attn[c]   = 1 / (1 + dots)
  y         = (-(w * Pi_bcast) * attn_bcast) @ W_out.T + b_out

On-chip layout: w stored column-major [c(part), n(free)] as 8 tiles
[128, 4096] bf16, so every sum_n is a free-axis reduce and both big
matmuls (f32r, full PE rate) need no big transposes beyond x itself
(PE-transposed per chunk).
"""

import sys

sys.path.insert(0, "/opt/trn_rl_repo")

import numpy as np
import concourse.bacc as bacc
import concourse.tile as tile
from concourse import mybir
from concourse.bass_utils import run_bass_kernel_spmd
from concourse.masks import make_identity

F32 = mybir.dt.float32
F32R = mybir.dt.float32r
BF16 = mybir.dt.bfloat16
MUL = mybir.AluOpType.mult
ADD = mybir.AluOpType.add

B, N, D = 8, 4096, 1024
H, HD = 16, 64
P = 128
NT = D // P          # 8 col-partition tiles
CH = 512             # n-chunk
NCH = N // CH        # 8 chunks
MS = CH // P         # 4 n-subtiles per chunk


def build(reps=1):
    nc = bacc.Bacc()
    x_t = nc.dram_tensor("x", [N, D], F32, kind="ExternalInput")
    wq_t = nc.dram_tensor("wqT", [D, D], F32R, kind="ExternalInput")    # W_qkv.T
    wo_t = nc.dram_tensor("woT", [D, D], F32R, kind="ExternalInput")    # W_out.T
    temp_t = nc.dram_tensor("temp", [H, 1], F32, kind="ExternalInput")
    sel_t = nc.dram_tensor("sel", [NT, H, P], F32, kind="ExternalInput")
    selr_t = nc.dram_tensor("selr", [NT, H, P], F32R, kind="ExternalInput")
    bias_t = nc.dram_tensor("bout", [1, D], F32R, kind="ExternalInput")
    y_t = nc.dram_tensor("y", [N, D], F32, kind="ExternalOutput")

    with tile.TileContext(nc) as tc:
      for _rep in range(reps):
        with (
            tc.tile_pool(name="consts", bufs=1) as consts,
            tc.tile_pool(name="wmat", bufs=1) as wmat,
            tc.tile_pool(name="wsb", bufs=1) as wsb,
            tc.tile_pool(name="small", bufs=1) as small,
        ):
            # ---------- constants ----------
            ident = consts.tile([P, P], F32)
            make_identity(nc, ident)
            temp_sb = consts.tile([H, 1], F32)
            nc.sync.dma_start(out=temp_sb, in_=temp_t[:, :])
            bias_r = consts.tile([1, D], F32R)
            nc.sync.dma_start(out=bias_r, in_=bias_t[:, :])
            ones16 = consts.tile([H, 1], F32)
            nc.vector.memset(ones16, 1.0)
            ones1x16 = consts.tile([1, H], F32)
            nc.vector.memset(ones1x16, 1.0)
            ones1x128 = consts.tile([1, P], F32)
            nc.vector.memset(ones1x128, 1.0)
            ones1x128_r = consts.tile([1, P], F32R)
            nc.vector.tensor_copy(ones1x128_r, ones1x128)
            ones1x16r = consts.tile([1, H], F32R)
            nc.vector.tensor_copy(ones1x16r, ones1x16)

            # per-tile selectors (host constant): Sel01[t][j, p] = 1 iff j == 2t + p//64
            sel_f32 = []
            sel_r = []
            for t in range(NT):
                sf = consts.tile([H, P], F32, tag=f"self{t}", name=f"self{t}")
                nc.sync.dma_start(out=sf, in_=sel_t[t, :, :])
                sr = consts.tile([H, P], F32R, tag=f"selr{t}", name=f"selr{t}")
                nc.sync.dma_start(out=sr, in_=selr_t[t, :, :])
                sel_f32.append(sf)
                sel_r.append(sr)

            # weights (host pre-transposed); wq and wo share one slot (bufs=1):
            # wo is DMA'd after phase A frees wq
            wq_sb = wmat.tile([P, NT, D], F32R, tag="wm")
            for k in range(NT):
                nc.sync.dma_start(out=wq_sb[:, k, :], in_=wq_t[k * P : (k + 1) * P, :])

            # persistent big tensors
            w_tiles = [wsb.tile([P, N], BF16, tag=f"w{t}", name=f"w{t}") for t in range(NT)]
            s_strip = [small.tile([P, NCH], F32, tag=f"ss{t}", name=f"ss{t}") for t in range(NT)]
            d_strip = [small.tile([P, NCH], F32, tag=f"ds{t}", name=f"ds{t}") for t in range(NT)]

            # ---------- phase A: w = x @ WqkvT, s = sum_n w^2 ----------
            with (
                tc.tile_pool(name="achunk", bufs=2) as achunk,
                tc.tile_pool(name="scrA", bufs=3) as scrA,
                tc.tile_pool(name="psA", bufs=2, space="PSUM") as psA,
                tc.tile_pool(name="psTP", bufs=2, space="PSUM") as psTP,
            ):
                for c in range(NCH):
                    cs = slice(c * CH, (c + 1) * CH)
                    x_raw = achunk.tile([P, MS, D], F32, tag="xraw")
                    nc.sync.dma_start(
                        out=x_raw,
                        in_=x_t[cs, :].rearrange("(m p) i -> p m i", p=P),
                    )
                    xT = achunk.tile([P, NT, CH], F32R, tag="xT", bufs=1)
                    for k in range(NT):
                        tp_ps = psTP.tile([P, CH], F32, tag="tp")
                        for m in range(MS):
                            nc.tensor.transpose(
                                tp_ps[:, m * P : (m + 1) * P],
                                x_raw[:, m, k * P : (k + 1) * P],
                                ident,
                            )
                        nc.scalar.copy(out=xT[:, k, :], in_=tp_ps)
                    for t in range(NT):
                        w_ps = psA.tile([P, CH], F32, tag="mm1")
                        for k in range(NT):
                            nc.tensor.matmul(
                                w_ps,
                                wq_sb[:, k, t * P : (t + 1) * P],
                                xT[:, k, :],
                                start=(k == 0),
                                stop=(k == NT - 1),
                            )
                        nc.scalar.copy(out=w_tiles[t][:, cs], in_=w_ps)
                        junk = scrA.tile([P, CH], BF16, tag="junkA")
                        nc.vector.scalar_tensor_tensor(
                            out=junk,
                            in0=w_tiles[t][:, cs],
                            scalar=1.0,
                            in1=w_tiles[t][:, cs],
                            op0=MUL,
                            op1=MUL,
                            accum_out=s_strip[t][:, c : c + 1],
                        )

            # W_out.T load (reuses wq's slot; overlaps softmax phases)
            wo_sb = wmat.tile([P, NT, D], F32R, tag="wm")
            for k in range(NT):
                nc.sync.dma_start(out=wo_sb[:, k, :], in_=wo_t[k * P : (k + 1) * P, :])

            # softmax-side pool opens only after phase A frees its space
            soft = tc.alloc_tile_pool(name="soft", bufs=1)

            # ---------- stats 1: inv_temp, L_big ----------
            lbig = []
            with tc.tile_pool(name="psS1", bufs=2, space="PSUM") as psS1:
                for t in range(NT):
                    s_all = small.tile([P, 1], F32, tag=f"sall{t}")
                    nc.vector.reduce_sum(s_all, s_strip[t], axis=mybir.AxisListType.X)
                    nc.vector.tensor_scalar_max(out=s_all, in0=s_all, scalar1=1e-24)
                    rcp = small.tile([P, 1], F32, tag=f"rcp{t}")
                    nc.vector.reciprocal(rcp, s_all)
                    tb_ps = psS1.tile([P, 1], F32, tag="tb")
                    nc.tensor.matmul(tb_ps, sel_f32[t], temp_sb, start=True, stop=True)
                    inv_t = small.tile([P, 1], F32, tag=f"invt{t}")
                    nc.vector.tensor_mul(inv_t, rcp, tb_ps)
                    lb = small.tile([P, H], BF16, tag=f"lbig{t}")
                    nc.vector.memset(lb, 0.0)
                    nc.vector.tensor_copy(lb[0:HD, 2 * t : 2 * t + 1], inv_t[0:HD, :])
                    nc.vector.tensor_copy(
                        lb[HD:P, 2 * t + 1 : 2 * t + 2], inv_t[HD:P, :]
                    )
                    lbig.append(lb)

            # ---------- phase L: logits[h, n] (squares on DVE, copies on ACT) ----------
            logits = soft.tile([H, N], F32R, tag="logits")
            with (
                tc.tile_pool(name="scrL", bufs=3) as scrL,
                tc.tile_pool(name="psL", bufs=2, space="PSUM") as psL,
            ):
                for c in range(NCH):
                    cs = slice(c * CH, (c + 1) * CH)
                    lg_ps = psL.tile([H, CH], F32, tag="lg")
                    for t in range(NT):
                        w2t = scrL.tile([P, CH], BF16, tag="w2t")
                        nc.vector.tensor_mul(
                            w2t, w_tiles[t][:, cs], w_tiles[t][:, cs]
                        )
                        nc.tensor.matmul(
                            lg_ps, lbig[t], w2t, start=(t == 0), stop=(t == NT - 1)
                        )
                    nc.scalar.copy(out=logits[:, cs], in_=lg_ps)

            # ---------- softmax over h, log-sum-exp form ----------
            # Pi = exp(logits - ln(sum_h exp(logits))); avoids a 4096-wide
            # iterative reciprocal on DVE.
            epool = tc.alloc_tile_pool(name="epool", bufs=1)
            e_hn = epool.tile([H, N], F32R, tag="ehn")
            nc.scalar.activation(
                out=e_hn, in_=logits.bitcast(F32),
                func=mybir.ActivationFunctionType.Exp,
            )
            ones16r = consts.tile([H, 1], F32R)
            nc.vector.tensor_copy(ones16r, ones16)
            # Pi overwrites logits in place (exp(logits - lnS) reads+writes same tile)
            pi_hn = logits
            sume_row = small.tile([1, N], F32, tag="sumerow")
            lns_row = small.tile([1, N], F32R, tag="lnsrow")
            with tc.tile_pool(name="psSM", bufs=2, space="PSUM") as psSM:
                for c in range(NCH):
                    cs = slice(c * CH, (c + 1) * CH)
                    se_ps = psSM.tile([1, CH], F32, tag="se")
                    nc.tensor.matmul(
                        se_ps, ones16r, e_hn[:, cs], start=True, stop=True
                    )
                    nc.scalar.copy(out=sume_row[:, cs], in_=se_ps)
                nc.scalar.activation(
                    out=lns_row, in_=sume_row, func=mybir.ActivationFunctionType.Ln
                )
                for c in range(NCH):
                    cs = slice(c * CH, (c + 1) * CH)
                    lnb_ps = psSM.tile([H, CH], F32, tag="lnb")
                    nc.tensor.matmul(
                        lnb_ps, ones1x16r, lns_row[:, cs], start=True, stop=True
                    )
                    nc.vector.tensor_sub(
                        logits[:, cs], logits[:, cs].bitcast(F32), lnb_ps
                    )
                    nc.scalar.activation(
                        out=pi_hn[:, cs],
                        in_=logits[:, cs].bitcast(F32),
                        func=mybir.ActivationFunctionType.Exp,
                    )

            epool.release()

            sumpi = small.tile([H, 1], F32, tag="sumpi")
            nc.vector.reduce_sum(
                sumpi, pi_hn.bitcast(F32), axis=mybir.AxisListType.X
            )
            nc.vector.tensor_scalar_add(out=sumpi, in0=sumpi, scalar1=1e-8)
            ispi = small.tile([H, 1], F32, tag="ispi")
            nc.vector.reciprocal(ispi, sumpi)

            # ---------- phase B (merged): u = w*Pi_b overwrites w; dots ----------
            # 1024-wide chunks to amortize fixed per-op costs
            CHB = 1024
            NCHB = N // CHB
            with (
                tc.tile_pool(name="scrB", bufs=3) as scrB,
                tc.tile_pool(name="psB1", bufs=2, space="PSUM") as psB1,
            ):
                for c in range(NCHB):
                    cs = slice(c * CHB, (c + 1) * CHB)
                    for t in range(NT):
                        pib_ps = psB1.tile([P, CHB], F32, tag="pib")
                        for hh in range(CHB // CH):
                            nc.tensor.matmul(
                                pib_ps[:, hh * CH : (hh + 1) * CH],
                                sel_r[t],
                                pi_hn[:, c * CHB + hh * CH : c * CHB + (hh + 1) * CH],
                                start=True,
                                stop=True,
                            )
                        pib_sb = scrB.tile([P, CHB], BF16, tag="pibsb")
                        nc.scalar.copy(out=pib_sb, in_=pib_ps)
                        u_tmp = scrB.tile([P, CHB], BF16, tag="utmp")
                        nc.vector.tensor_mul(u_tmp, w_tiles[t][:, cs], pib_sb)
                        junk = scrB.tile([P, CHB], BF16, tag="junkB")
                        nc.vector.scalar_tensor_tensor(
                            out=junk,
                            in0=u_tmp,
                            scalar=1.0,
                            in1=w_tiles[t][:, cs],
                            op0=MUL,
                            op1=MUL,
                            accum_out=d_strip[t][:, c : c + 1],
                        )
                        # u overwrites w in place (WAR on the STT above)
                        nc.gpsimd.tensor_copy(out=w_tiles[t][:, cs], in_=u_tmp)

            # ---------- stats 2: attn; W' = -attn * WoutT (bf16) ----------
            wob, _wob_free = tc.tile([P, NT, D], BF16, name="wob")
            bias_bf = consts.tile([1, D], BF16)
            nc.vector.tensor_copy(bias_bf, bias_r.bitcast(F32))
            ones1x128b = consts.tile([1, P], BF16)
            nc.vector.memset(ones1x128b, 1.0)
            with tc.tile_pool(name="psS2", bufs=2, space="PSUM") as psS2:
                for t in range(NT):
                    isp_ps = psS2.tile([P, 1], F32, tag="isp")
                    nc.tensor.matmul(isp_ps, sel_f32[t], ispi, start=True, stop=True)
                    dots = small.tile([P, 1], F32, tag=f"dots{t}")
                    nc.vector.reduce_sum(
                        dots, d_strip[t][:, 0 : N // 1024], axis=mybir.AxisListType.X
                    )
                    nc.vector.tensor_mul(dots, dots, isp_ps)
                    nc.vector.tensor_scalar_add(out=dots, in0=dots, scalar1=1.0)
                    attn = small.tile([P, 1], F32, tag=f"attn{t}")
                    nc.vector.reciprocal(attn, dots)
                    nc.vector.tensor_scalar_mul(out=attn, in0=attn, scalar1=-1.0)
                    nc.vector.tensor_scalar_mul(
                        out=wob[:, t, :],
                        in0=wo_sb[:, t, :].bitcast(F32),
                        scalar1=attn,
                    )

            # ---------- phase MM2: y = u.T @ W' + b (dense PE) ----------
            with (
                tc.tile_pool(name="och", bufs=1) as och,
                tc.tile_pool(name="psMM2", bufs=4, space="PSUM") as psMM2,
            ):
                for c in range(NCH):
                    cs = slice(c * CH, (c + 1) * CH)
                    outf = och.tile([P, MS, D], F32, tag="outf")
                    for m in range(MS):
                        ms_ = slice(c * CH + m * P, c * CH + (m + 1) * P)
                        for oh in range(2):
                            os_ = slice(oh * CH, (oh + 1) * CH)
                            f_ps = psMM2.tile([P, CH], F32, tag="mm2")
                            for t in range(NT):
                                nc.tensor.matmul(
                                    f_ps,
                                    w_tiles[t][:, ms_],
                                    wob[:, t, os_],
                                    start=(t == 0),
                                    stop=False,
                                )
                            nc.tensor.matmul(
                                f_ps,
                                ones1x128b,
                                bias_bf[:, os_],
                                start=False,
                                stop=True,
                            )
                            nc.scalar.copy(out=outf[:, m, os_], in_=f_ps)
                    nc.sync.dma_start(
                        out=y_t[cs, :].rearrange("(m p) i -> p m i", p=P),
                        in_=outf,
                    )
            _wob_free()
            soft.release()

    if not nc.is_finalized():
        nc.finalize()
    return nc


_NC_CACHE = None
_LAST_IN_MAPS = None
_RUNNER = None


def _make_runner(nc, n_cores):
    """Like bass2jax.run_bass_via_pjrt but with the jitted callable cached,
    so repeat calls don't re-trace/re-compile the XLA wrapper."""
    import jax
    from jax.experimental.shard_map import shard_map
    from jax.sharding import Mesh, PartitionSpec
    from concourse import mybir as _mybir
    from concourse.bass2jax import (
        _bass_exec_p,
        install_neuronx_cc_hook,
        partition_id_tensor,
    )

    install_neuronx_cc_hook()

    partition_name = nc.partition_id_tensor.name if nc.partition_id_tensor else None
    in_names, out_names, out_avals, zero_outs = [], [], [], []
    for alloc in nc.m.functions[0].allocations:
        if not isinstance(alloc, _mybir.MemoryLocationSet):
            continue
        name = alloc.memorylocations[0].name
        if alloc.kind == "ExternalInput":
            if name != partition_name:
                in_names.append(name)
        elif alloc.kind == "ExternalOutput":
            shape = tuple(alloc.tensor_shape)
            dtype = _mybir.dt.np(alloc.dtype)
            out_names.append(name)
            out_avals.append(jax.core.ShapedArray(shape, dtype))
            zero_outs.append(np.zeros(shape, dtype))
    n_params = len(in_names)
    n_outs = len(out_names)
    all_in_names = in_names + out_names + (
        [partition_name] if partition_name else []
    )
    donate = tuple(range(n_params, n_params + n_outs))

    def _body(*args):
        operands = list(args)
        if partition_name is not None:
            operands.append(partition_id_tensor())
        outs = _bass_exec_p.bind(
            *operands,
            out_avals=tuple(out_avals),
            in_names=tuple(all_in_names),
            out_names=tuple(out_names),
            lowering_input_output_aliases=(),
            sim_require_finite=True,
            sim_require_nnan=True,
            nc=nc,
        )
        return tuple(outs)

    devices = jax.devices()[:n_cores]
    mesh = Mesh(np.asarray(devices), ("core",))
    in_specs = (PartitionSpec("core"),) * (n_params + n_outs)
    out_specs = (PartitionSpec("core"),) * n_outs
    sharded = jax.jit(
        shard_map(
            _body, mesh=mesh, in_specs=in_specs, out_specs=out_specs, check_rep=False
        ),
        donate_argnums=donate,
        keep_unused=True,
    )

    def run(in_maps):
        concat_in = [
            np.concatenate([np.asarray(m[name]) for m in in_maps], axis=0)
            for name in in_names
        ]
        concat_zeros = [
            np.zeros((n_cores * z.shape[0], *z.shape[1:]), z.dtype)
            for z in zero_outs
        ]
        out_arrs = sharded(*concat_in, *concat_zeros)
        return {
            name: np.asarray(out_arrs[i]).reshape(n_cores, *out_avals[i].shape)
            for i, name in enumerate(out_names)
        }

    run.sharded = sharded
    run.meta = (in_names, out_names, out_avals, n_params, n_outs)
    return run


def kernel(x, W_qkv, temp, W_out, b_out):
    global _NC_CACHE, _RUNNER
    if _NC_CACHE is None:
        _NC_CACHE = build()
        _RUNNER = _make_runner(_NC_CACHE, B)
    nc = _NC_CACHE

    x = np.asarray(x, dtype=np.float32)
    wqT = np.ascontiguousarray(np.asarray(W_qkv, dtype=np.float32).T)
    woT = np.ascontiguousarray(np.asarray(W_out, dtype=np.float32).T)
    temp = np.ascontiguousarray(np.asarray(temp, dtype=np.float32).reshape(H, 1))
    bout = np.ascontiguousarray(np.asarray(b_out, dtype=np.float32).reshape(1, D))

    sel = np.zeros((NT, H, P), dtype=np.float32)
    for t in range(NT):
        sel[t, 2 * t, 0:HD] = 1.0
        sel[t, 2 * t + 1, HD:P] = 1.0

    core_ids = list(range(B))
    in_maps = [
        {"x": np.ascontiguousarray(x[i]), "wqT": wqT, "woT": woT,
         "temp": temp, "bout": bout, "sel": sel, "selr": sel}
        for i in core_ids
    ]
    global _LAST_IN_MAPS
    _LAST_IN_MAPS = in_maps
    out = _RUNNER(in_maps)
    return out["y"]


if __name__ == "__main__":
    rng = np.random.default_rng(0)
    x = rng.standard_normal((B, N, D), dtype=np.float32)
    W_qkv = (rng.standard_normal((D, D), dtype=np.float32) * 0.02).astype(np.float32)
    temp = np.ones((H, 1), dtype=np.float32)
    W_out = (rng.standard_normal((D, D), dtype=np.float32) * 0.02).astype(np.float32)
    b_out = np.zeros((D,), dtype=np.float32)
    y = kernel(x=x, W_qkv=W_qkv, temp=temp, W_out=W_out, b_out=b_out)
    print("kernel ran, y shape", y.shape, "mean abs", np.abs(y).mean())


# revision 30
# speedup vs baseline: 20260.9796x; 1.4608x over previous
"""AttentionTSSA Trainium2 kernel.

Sharding: data-parallel over batch. B=8 -> one batch element per NeuronCore,
zero collectives. Host slices inputs / stacks outputs.

Per-core math (x: [N=4096, D=1024], heads h=16, head dim d=64):
  w[n, c]   = x @ W_qkv.T                   (c = hd flattened head*64+dd)
  s[c]      = sum_n w^2                     (col norms squared)
  logits[h,n] = temp[h] * sum_dd w^2[hd,n] / max(s[hd], 1e-24)
  Pi        = softmax_h(logits)
  dots[c]   = (sum_n Pi[h,n] * w^2[c,n]) / (sum_n Pi[h,n] + 1e-8)
  attn[c]   = 1 / (1 + dots)
  y         = (-(w * Pi_bcast) * attn_bcast) @ W_out.T + b_out

On-chip layout: w stored column-major [c(part), n(free)] as 8 tiles
[128, 4096] bf16, so every sum_n is a free-axis reduce and both big
matmuls (f32r, full PE rate) need no big transposes beyond x itself
(PE-transposed per chunk).
"""

import sys

sys.path.insert(0, "/opt/trn_rl_repo")

import numpy as np
import concourse.bacc as bacc
import concourse.tile as tile
from concourse import mybir
from concourse.bass_utils import run_bass_kernel_spmd
from concourse.masks import make_identity

F32 = mybir.dt.float32
F32R = mybir.dt.float32r
BF16 = mybir.dt.bfloat16
MUL = mybir.AluOpType.mult
ADD = mybir.AluOpType.add

B, N, D = 8, 4096, 1024
H, HD = 16, 64
P = 128
NT = D // P          # 8 col-partition tiles
CH = 512             # n-chunk
NCH = N // CH        # 8 chunks
MS = CH // P         # 4 n-subtiles per chunk


def build(reps=1, phases="ALSBM"):
    nc = bacc.Bacc()
    x_t = nc.dram_tensor("x", [N, D], F32, kind="ExternalInput")
    wq_t = nc.dram_tensor("wqT", [D, D], F32R, kind="ExternalInput")    # W_qkv.T
    wo_t = nc.dram_tensor("woT", [D, D], F32R, kind="ExternalInput")    # W_out.T
    temp_t = nc.dram_tensor("temp", [H, 1], F32, kind="ExternalInput")
    sel_t = nc.dram_tensor("sel", [NT, H, P], F32, kind="ExternalInput")
    selr_t = nc.dram_tensor("selr", [NT, H, P], F32R, kind="ExternalInput")
    bias_t = nc.dram_tensor("bout", [1, D], F32R, kind="ExternalInput")
    y_t = nc.dram_tensor("y", [N, D], F32, kind="ExternalOutput")

    with tile.TileContext(nc) as tc:
      for _rep in range(reps):
        with (
            tc.tile_pool(name="consts", bufs=1) as consts,
            tc.tile_pool(name="wmat", bufs=1) as wmat,
            tc.tile_pool(name="wsb", bufs=1) as wsb,
            tc.tile_pool(name="small", bufs=1) as small,
        ):
            # ---------- constants ----------
            ident = consts.tile([P, P], F32)
            make_identity(nc, ident)
            temp_sb = consts.tile([H, 1], F32)
            nc.sync.dma_start(out=temp_sb, in_=temp_t[:, :])
            bias_r = consts.tile([1, D], F32R)
            nc.sync.dma_start(out=bias_r, in_=bias_t[:, :])
            ones16 = consts.tile([H, 1], F32)
            nc.vector.memset(ones16, 1.0)
            ones1x16 = consts.tile([1, H], F32)
            nc.vector.memset(ones1x16, 1.0)
            ones1x128 = consts.tile([1, P], F32)
            nc.vector.memset(ones1x128, 1.0)
            ones1x128_r = consts.tile([1, P], F32R)
            nc.vector.tensor_copy(ones1x128_r, ones1x128)
            ones1x16r = consts.tile([1, H], F32R)
            nc.vector.tensor_copy(ones1x16r, ones1x16)

            # per-tile selectors (host constant): Sel01[t][j, p] = 1 iff j == 2t + p//64
            sel_f32 = []
            sel_r = []
            for t in range(NT):
                sf = consts.tile([H, P], F32, tag=f"self{t}", name=f"self{t}")
                nc.sync.dma_start(out=sf, in_=sel_t[t, :, :])
                sr = consts.tile([H, P], F32R, tag=f"selr{t}", name=f"selr{t}")
                nc.sync.dma_start(out=sr, in_=selr_t[t, :, :])
                sel_f32.append(sf)
                sel_r.append(sr)

            # weights (host pre-transposed); wq and wo share one slot (bufs=1):
            # wo is DMA'd after phase A frees wq
            wq_sb = wmat.tile([P, NT, D], F32R, tag="wm")
            for k in range(NT):
                nc.sync.dma_start(out=wq_sb[:, k, :], in_=wq_t[k * P : (k + 1) * P, :])

            # persistent big tensors
            w_tiles = [wsb.tile([P, N], BF16, tag=f"w{t}", name=f"w{t}") for t in range(NT)]
            s_strip = [small.tile([P, NCH], F32, tag=f"ss{t}", name=f"ss{t}") for t in range(NT)]
            d_strip = [small.tile([P, NCH], F32, tag=f"ds{t}", name=f"ds{t}") for t in range(NT)]

            # ---------- phase A: w = x @ WqkvT, s = sum_n w^2 ----------
            with (
                tc.tile_pool(name="achunk", bufs=2) as achunk,
                tc.tile_pool(name="scrA", bufs=3) as scrA,
                tc.tile_pool(name="psA", bufs=2, space="PSUM") as psA,
                tc.tile_pool(name="psTP", bufs=2, space="PSUM") as psTP,
            ):
                for c in range(NCH):
                    cs = slice(c * CH, (c + 1) * CH)
                    x_raw = achunk.tile([P, MS, D], F32, tag="xraw")
                    nc.sync.dma_start(
                        out=x_raw,
                        in_=x_t[cs, :].rearrange("(m p) i -> p m i", p=P),
                    )
                    xT = achunk.tile([P, NT, CH], F32R, tag="xT", bufs=1)
                    for k in range(NT):
                        tp_ps = psTP.tile([P, CH], F32, tag="tp")
                        for m in range(MS):
                            nc.tensor.transpose(
                                tp_ps[:, m * P : (m + 1) * P],
                                x_raw[:, m, k * P : (k + 1) * P],
                                ident,
                            )
                        nc.scalar.copy(out=xT[:, k, :], in_=tp_ps)
                    for t in range(NT):
                        w_ps = psA.tile([P, CH], F32, tag="mm1")
                        for k in range(NT):
                            nc.tensor.matmul(
                                w_ps,
                                wq_sb[:, k, t * P : (t + 1) * P],
                                xT[:, k, :],
                                start=(k == 0),
                                stop=(k == NT - 1),
                            )
                        nc.scalar.copy(out=w_tiles[t][:, cs], in_=w_ps)
                        junk = scrA.tile([P, CH], BF16, tag="junkA")
                        nc.vector.scalar_tensor_tensor(
                            out=junk,
                            in0=w_tiles[t][:, cs],
                            scalar=1.0,
                            in1=w_tiles[t][:, cs],
                            op0=MUL,
                            op1=MUL,
                            accum_out=s_strip[t][:, c : c + 1],
                        )

            # W_out.T load (reuses wq's slot; overlaps softmax phases)
            wo_sb = wmat.tile([P, NT, D], F32R, tag="wm")
            for k in range(NT):
                nc.sync.dma_start(out=wo_sb[:, k, :], in_=wo_t[k * P : (k + 1) * P, :])

            # softmax-side pool opens only after phase A frees its space
            soft = tc.alloc_tile_pool(name="soft", bufs=1)

            # ---------- stats 1: inv_temp, L_big ----------
            lbig = []
            with tc.tile_pool(name="psS1", bufs=2, space="PSUM") as psS1:
                for t in range(NT):
                    s_all = small.tile([P, 1], F32, tag=f"sall{t}")
                    nc.vector.reduce_sum(s_all, s_strip[t], axis=mybir.AxisListType.X)
                    nc.vector.tensor_scalar_max(out=s_all, in0=s_all, scalar1=1e-24)
                    rcp = small.tile([P, 1], F32, tag=f"rcp{t}")
                    nc.vector.reciprocal(rcp, s_all)
                    tb_ps = psS1.tile([P, 1], F32, tag="tb")
                    nc.tensor.matmul(tb_ps, sel_f32[t], temp_sb, start=True, stop=True)
                    inv_t = small.tile([P, 1], F32, tag=f"invt{t}")
                    nc.vector.tensor_mul(inv_t, rcp, tb_ps)
                    lb = small.tile([P, H], BF16, tag=f"lbig{t}")
                    nc.vector.memset(lb, 0.0)
                    nc.vector.tensor_copy(lb[0:HD, 2 * t : 2 * t + 1], inv_t[0:HD, :])
                    nc.vector.tensor_copy(
                        lb[HD:P, 2 * t + 1 : 2 * t + 2], inv_t[HD:P, :]
                    )
                    lbig.append(lb)

            # ---------- phase L: logits[h, n] (squares on DVE, copies on ACT) ----------
            if "L" not in phases:
                for t in range(NT):
                    nc.gpsimd.dma_start(
                        out=y_t[t * CH : (t + 1) * CH, :].rearrange(
                            "(m p) i -> p m i", p=P
                        ),
                        in_=w_tiles[t].rearrange("p (m i) -> p m i", i=D),
                    )
                nc.sync.dma_start(out=y_t[0:P, 0:NCH], in_=s_strip[0])
                soft.release()
                continue
            logits = soft.tile([H, N], F32R, tag="logits")
            with (
                tc.tile_pool(name="scrL", bufs=3) as scrL,
                tc.tile_pool(name="psL", bufs=2, space="PSUM") as psL,
            ):
                for c in range(NCH):
                    cs = slice(c * CH, (c + 1) * CH)
                    lg_ps = psL.tile([H, CH], F32, tag="lg")
                    for t in range(NT):
                        w2t = scrL.tile([P, CH], BF16, tag="w2t")
                        nc.vector.tensor_mul(
                            w2t, w_tiles[t][:, cs], w_tiles[t][:, cs]
                        )
                        nc.tensor.matmul(
                            lg_ps, lbig[t], w2t, start=(t == 0), stop=(t == NT - 1)
                        )
                    nc.scalar.copy(out=logits[:, cs], in_=lg_ps)

            # ---------- softmax over h, log-sum-exp form ----------
            if "S" not in phases:
                nc.sync.dma_start(
                    out=y_t[0 : H * MS, :].rearrange("(m p) i -> p m i", p=H),
                    in_=logits.bitcast(F32).rearrange("p (m i) -> p m i", i=D),
                )
                soft.release()
                continue
            # Pi = exp(logits - ln(sum_h exp(logits))); avoids a 4096-wide
            # iterative reciprocal on DVE.
            epool = tc.alloc_tile_pool(name="epool", bufs=1)
            e_hn = epool.tile([H, N], F32R, tag="ehn")
            nc.scalar.activation(
                out=e_hn, in_=logits.bitcast(F32),
                func=mybir.ActivationFunctionType.Exp,
            )
            ones16r = consts.tile([H, 1], F32R)
            nc.vector.tensor_copy(ones16r, ones16)
            # Pi overwrites logits in place (exp(logits - lnS) reads+writes same tile)
            pi_hn = logits
            sume_row = small.tile([1, N], F32, tag="sumerow")
            lns_row = small.tile([1, N], F32R, tag="lnsrow")
            # Few wide ops instead of many small per-chunk ops: each
            # cross-engine hop costs ~1us of sync latency on HW.
            with tc.tile_pool(name="psSM", bufs=1, space="PSUM") as psSM:
                se_ps = psSM.tile([1, N], F32, tag="sm_big")
                for c in range(NCH):
                    cs = slice(c * CH, (c + 1) * CH)
                    nc.tensor.matmul(
                        se_ps[:, cs], ones16r, e_hn[:, cs], start=True, stop=True
                    )
                nc.scalar.copy(out=sume_row, in_=se_ps)
                nc.scalar.activation(
                    out=lns_row, in_=sume_row, func=mybir.ActivationFunctionType.Ln
                )
                lnb_ps = psSM.tile([H, N], F32, tag="sm_big")
                for c in range(NCH):
                    cs = slice(c * CH, (c + 1) * CH)
                    nc.tensor.matmul(
                        lnb_ps[:, cs], ones1x16r, lns_row[:, cs], start=True, stop=True
                    )
                nc.vector.tensor_sub(logits, logits.bitcast(F32), lnb_ps)
                nc.scalar.activation(
                    out=pi_hn,
                    in_=logits.bitcast(F32),
                    func=mybir.ActivationFunctionType.Exp,
                )

            epool.release()

            sumpi = small.tile([H, 1], F32, tag="sumpi")
            nc.vector.reduce_sum(
                sumpi, pi_hn.bitcast(F32), axis=mybir.AxisListType.X
            )
            nc.vector.tensor_scalar_add(out=sumpi, in0=sumpi, scalar1=1e-8)
            ispi = small.tile([H, 1], F32, tag="ispi")
            nc.vector.reciprocal(ispi, sumpi)

            # ---------- phase B (merged): u = w*Pi_b overwrites w; dots ----------
            if "B" not in phases:
                nc.sync.dma_start(
                    out=y_t[0 : H * MS, :].rearrange("(m p) i -> p m i", p=H),
                    in_=pi_hn.bitcast(F32).rearrange("p (m i) -> p m i", i=D),
                )
                nc.sync.dma_start(out=y_t[H * MS : H * MS + H, 0:1], in_=ispi)
                soft.release()
                continue
            # 1024-wide chunks to amortize fixed per-op costs
            CHB = 1024
            NCHB = N // CHB
            with (
                tc.tile_pool(name="scrB", bufs=4) as scrB,
                tc.tile_pool(name="psB1", bufs=3, space="PSUM") as psB1,
            ):
                for c in range(NCHB):
                    cs = slice(c * CHB, (c + 1) * CHB)
                    for t in range(NT):
                        pib_ps = psB1.tile([P, CHB], F32, tag="pib")
                        for hh in range(CHB // CH):
                            nc.tensor.matmul(
                                pib_ps[:, hh * CH : (hh + 1) * CH],
                                sel_r[t],
                                pi_hn[:, c * CHB + hh * CH : c * CHB + (hh + 1) * CH],
                                start=True,
                                stop=True,
                            )
                        pib_sb = scrB.tile([P, CHB], BF16, tag="pibsb")
                        nc.scalar.copy(out=pib_sb, in_=pib_ps)
                        u_tmp = scrB.tile([P, CHB], BF16, tag="utmp")
                        nc.vector.tensor_mul(u_tmp, w_tiles[t][:, cs], pib_sb)
                        junk = scrB.tile([P, CHB], BF16, tag="junkB")
                        nc.vector.scalar_tensor_tensor(
                            out=junk,
                            in0=u_tmp,
                            scalar=1.0,
                            in1=w_tiles[t][:, cs],
                            op0=MUL,
                            op1=MUL,
                            accum_out=d_strip[t][:, c : c + 1],
                        )
                        # u overwrites w in place (WAR on the STT above)
                        nc.gpsimd.tensor_copy(out=w_tiles[t][:, cs], in_=u_tmp)

            # ---------- stats 2: attn; W' = -attn * WoutT (bf16) ----------
            wob, _wob_free = tc.tile([P, NT, D], BF16, name="wob")
            bias_bf = consts.tile([1, D], BF16)
            nc.vector.tensor_copy(bias_bf, bias_r.bitcast(F32))
            ones1x128b = consts.tile([1, P], BF16)
            nc.vector.memset(ones1x128b, 1.0)
            with tc.tile_pool(name="psS2", bufs=2, space="PSUM") as psS2:
                for t in range(NT):
                    isp_ps = psS2.tile([P, 1], F32, tag="isp")
                    nc.tensor.matmul(isp_ps, sel_f32[t], ispi, start=True, stop=True)
                    dots = small.tile([P, 1], F32, tag=f"dots{t}")
                    nc.vector.reduce_sum(
                        dots, d_strip[t][:, 0 : N // 1024], axis=mybir.AxisListType.X
                    )
                    nc.vector.tensor_mul(dots, dots, isp_ps)
                    nc.vector.tensor_scalar_add(out=dots, in0=dots, scalar1=1.0)
                    attn = small.tile([P, 1], F32, tag=f"attn{t}")
                    nc.vector.reciprocal(attn, dots)
                    nc.vector.tensor_scalar_mul(out=attn, in0=attn, scalar1=-1.0)
                    nc.vector.tensor_scalar_mul(
                        out=wob[:, t, :],
                        in0=wo_sb[:, t, :].bitcast(F32),
                        scalar1=attn,
                    )

            # ---------- phase MM2: y = u.T @ W' + b (dense PE) ----------
            if "M" not in phases:
                for t in range(NT):
                    nc.gpsimd.dma_start(
                        out=y_t[t * CH : (t + 1) * CH, :].rearrange(
                            "(m p) i -> p m i", p=P
                        ),
                        in_=w_tiles[t].rearrange("p (m i) -> p m i", i=D),
                    )
                nc.gpsimd.dma_start(
                    out=y_t[0:D, :].rearrange("(t p) i -> p t i", p=P),
                    in_=wob,
                )
                _wob_free()
                soft.release()
                continue
            # bias materialized [128, D] once: bias broadcast over partitions
            bias_sb, _bias_free = tc.tile([P, D], F32, name="bias_sb")
            with tc.tile_pool(name="psBb", bufs=1, space="PSUM") as psBb:
                bb_ps = psBb.tile([P, D], F32, tag="bb")
                for oh in range(2):
                    os_ = slice(oh * CH, (oh + 1) * CH)
                    nc.tensor.matmul(
                        bb_ps[:, os_],
                        ones1x128b,
                        bias_bf[:, os_],
                        start=True,
                        stop=True,
                    )
                nc.scalar.copy(out=bias_sb, in_=bb_ps)

            # MM2: pure PE accumulation; PSUM evicted by DVE with fused
            # bias add; output double-buffered at half-chunk granularity
            with (
                tc.tile_pool(name="och", bufs=2) as och,
                tc.tile_pool(name="psMM2", bufs=4, space="PSUM") as psMM2,
            ):
                for c in range(NCH):
                    for half in range(2):
                        outf = och.tile([P, MS // 2, D], F32, tag="outf")
                        for mh in range(MS // 2):
                            m = half * (MS // 2) + mh
                            ms_ = slice(c * CH + m * P, c * CH + (m + 1) * P)
                            for oh in range(2):
                                os_ = slice(oh * CH, (oh + 1) * CH)
                                f_ps = psMM2.tile([P, CH], F32, tag="mm2")
                                for t in range(NT):
                                    nc.tensor.matmul(
                                        f_ps,
                                        w_tiles[t][:, ms_],
                                        wob[:, t, os_],
                                        start=(t == 0),
                                        stop=(t == NT - 1),
                                    )
                                nc.vector.scalar_tensor_tensor(
                                    out=outf[:, mh, os_],
                                    in0=f_ps,
                                    scalar=1.0,
                                    in1=bias_sb[:, os_],
                                    op0=MUL,
                                    op1=ADD,
                                )
                        nc.sync.dma_start(
                            out=y_t[
                                c * CH + half * CH // 2 : c * CH + (half + 1) * CH // 2,
                                :,
                            ].rearrange("(m p) i -> p m i", p=P),
                            in_=outf,
                        )
            _bias_free()
            _wob_free()
            soft.release()

    if not nc.is_finalized():
        nc.finalize()
    return nc


_NC_CACHE = None
_LAST_IN_MAPS = None
_RUNNER = None


def _make_runner(nc, n_cores):
    """Like bass2jax.run_bass_via_pjrt but with the jitted callable cached,
    so repeat calls don't re-trace/re-compile the XLA wrapper."""
    import jax
    from jax.experimental.shard_map import shard_map
    from jax.sharding import Mesh, PartitionSpec
    from concourse import mybir as _mybir
    from concourse.bass2jax import (
        _bass_exec_p,
        install_neuronx_cc_hook,
        partition_id_tensor,
    )

    install_neuronx_cc_hook()

    partition_name = nc.partition_id_tensor.name if nc.partition_id_tensor else None
    in_names, out_names, out_avals, zero_outs = [], [], [], []
    for alloc in nc.m.functions[0].allocations:
        if not isinstance(alloc, _mybir.MemoryLocationSet):
            continue
        name = alloc.memorylocations[0].name
        if alloc.kind == "ExternalInput":
            if name != partition_name:
                in_names.append(name)
        elif alloc.kind == "ExternalOutput":
            shape = tuple(alloc.tensor_shape)
            dtype = _mybir.dt.np(alloc.dtype)
            out_names.append(name)
            out_avals.append(jax.core.ShapedArray(shape, dtype))
            zero_outs.append(np.zeros(shape, dtype))
    n_params = len(in_names)
    n_outs = len(out_names)
    all_in_names = in_names + out_names + (
        [partition_name] if partition_name else []
    )
    donate = tuple(range(n_params, n_params + n_outs))

    def _body(*args):
        operands = list(args)
        if partition_name is not None:
            operands.append(partition_id_tensor())
        outs = _bass_exec_p.bind(
            *operands,
            out_avals=tuple(out_avals),
            in_names=tuple(all_in_names),
            out_names=tuple(out_names),
            lowering_input_output_aliases=(),
            sim_require_finite=True,
            sim_require_nnan=True,
            nc=nc,
        )
        return tuple(outs)

    devices = jax.devices()[:n_cores]
    mesh = Mesh(np.asarray(devices), ("core",))
    in_specs = (PartitionSpec("core"),) * (n_params + n_outs)
    out_specs = (PartitionSpec("core"),) * n_outs
    sharded = jax.jit(
        shard_map(
            _body, mesh=mesh, in_specs=in_specs, out_specs=out_specs, check_rep=False
        ),
        donate_argnums=donate,
        keep_unused=True,
    )

    def run(in_maps):
        concat_in = [
            np.concatenate([np.asarray(m[name]) for m in in_maps], axis=0)
            for name in in_names
        ]
        concat_zeros = [
            np.zeros((n_cores * z.shape[0], *z.shape[1:]), z.dtype)
            for z in zero_outs
        ]
        out_arrs = sharded(*concat_in, *concat_zeros)
        return {
            name: np.asarray(out_arrs[i]).reshape(n_cores, *out_avals[i].shape)
            for i, name in enumerate(out_names)
        }

    run.sharded = sharded
    run.meta = (in_names, out_names, out_avals, n_params, n_outs)
    return run


def kernel(x, W_qkv, temp, W_out, b_out):
    global _NC_CACHE, _RUNNER
    if _NC_CACHE is None:
        _NC_CACHE = build()
        _RUNNER = _make_runner(_NC_CACHE, B)
    nc = _NC_CACHE

    x = np.asarray(x, dtype=np.float32)
    wqT = np.ascontiguousarray(np.asarray(W_qkv, dtype=np.float32).T)
    woT = np.ascontiguousarray(np.asarray(W_out, dtype=np.float32).T)
    temp = np.ascontiguousarray(np.asarray(temp, dtype=np.float32).reshape(H, 1))
    bout = np.ascontiguousarray(np.asarray(b_out, dtype=np.float32).reshape(1, D))

    sel = np.zeros((NT, H, P), dtype=np.float32)
    for t in range(NT):
        sel[t, 2 * t, 0:HD] = 1.0
        sel[t, 2 * t + 1, HD:P] = 1.0

    core_ids = list(range(B))
    in_maps = [
        {"x": np.ascontiguousarray(x[i]), "wqT": wqT, "woT": woT,
         "temp": temp, "bout": bout, "sel": sel, "selr": sel}
        for i in core_ids
    ]
    global _LAST_IN_MAPS
    _LAST_IN_MAPS = in_maps
    out = _RUNNER(in_maps)
    return out["y"]


if __name__ == "__main__":
    rng = np.random.default_rng(0)
    x = rng.standard_normal((B, N, D), dtype=np.float32)
    W_qkv = (rng.standard_normal((D, D), dtype=np.float32) * 0.02).astype(np.float32)
    temp = np.ones((H, 1), dtype=np.float32)
    W_out = (rng.standard_normal((D, D), dtype=np.float32) * 0.02).astype(np.float32)
    b_out = np.zeros((D,), dtype=np.float32)
    y = kernel(x=x, W_qkv=W_qkv, temp=temp, W_out=W_out, b_out=b_out)
    print("kernel ran, y shape", y.shape, "mean abs", np.abs(y).mean())
